# revision 1
# baseline (speedup 1.0000x reference)
"""Distributed causal multi-head attention (Bass/Tile, 8 TRN2 NeuronCores).

Sharding: core = (batch b, head-group g) with b = core // 4, g = core % 4.
Each core owns 4 heads (two pairs) of batch b and computes their QKV from
the full x[b] locally -- no K/V collective at all.  After attention, one
AllGather per head pair redistributes the attention outputs (each core
keeps only its 512-row column block), and each core applies the full Wo
to its 512-row block.

Per core:
  q^T, k^T = (x @ Wq/Wk)^T  [128, 2048] per pair  (partition = head dims)
  v        =  x @ Wv        [128, 65] per (head, kchunk), ones col appended
  scores^T = k^T.T @ q^T    row-tiled pairs run concurrently on the PE
  softmax: full-width exp (scalar engine), causal mask as 0/1 post-mult,
  AV matmul with ones column -> unnormalized out^T + denominator row,
  normalize via reciprocal + PE broadcast + DVE multiply,
  AllGather (bf16) across the 4 head-group ranks, y = out^T.T @ Wo.

Pair-1's q^T/k^T projection matmuls are interleaved into pair-0's
attention emission to keep the PE dense (HAM clock-gate stays warm).
"""

import numpy as np

B, T, C, H = 2, 2048, 1024, 16
D = C // H            # 64
G = 4                 # head-group ranks per batch
HPC = H // G          # 4 heads per core
NP = HPC // 2         # 2 head pairs per core
TOWN = T // G         # 512 output rows owned per core
NQG = T // 512        # 4 query groups of 512
NKC = T // 128        # 16 key chunks of 128
CC = C // 128         # 8 contraction chunks
WCOL = 3 * HPC * D    # 768 packed qkv columns per core
SCALE = 1.0 / 32.0    # 1/sqrt(C), folded into Wq on host

_cached_nc = None
last_result = None


def _load_phase(nc, P, mybir):
    from concourse.bass import ts

    F32, BF16 = mybir.dt.float32, mybir.dt.bfloat16

    mask = P["const_p"].tile([128, 128], BF16, tag="mask")
    nc.sync.dma_start(mask[:], P["mask_ext"][:])
    P["mask"] = mask
    ones = P["const_p"].tile([128, 64], F32, tag="ones")
    nc.vector.memset(ones[:], 1.0)
    P["ones"] = ones

    qs = [nc.sync, nc.scalar]
    xt_sb = P["x_p"].tile([128, CC * T], BF16, tag="xt")
    for cc in range(CC):
        qs[cc % 2].dma_start(
            xt_sb[:, cc * T : (cc + 1) * T], P["xt_ext"][ts(cc, 128), :]
        )
    P["xt_sb"] = xt_sb
    wqkv_sb = P["w_p"].tile([128, CC * WCOL], BF16, tag="wqkv")
    for cc in range(CC):
        qs[cc % 2].dma_start(
            wqkv_sb[:, cc * WCOL : (cc + 1) * WCOL], P["wqkv_ext"][ts(cc, 128), :]
        )
    P["wqkv_sb"] = wqkv_sb


def _load_wo(nc, P, mybir):
    from concourse.bass import ts

    BF16 = mybir.dt.bfloat16
    wo_sb = P["w_p"].tile([128, CC * C], BF16, tag="wo")
    for cc in range(CC):
        nc.sync.dma_start(wo_sb[:, cc * C : (cc + 1) * C], P["wo_ext"][ts(cc, 128), :])
    P["wo_sb"] = wo_sb


def _v_init(nc, P, mybir):
    """v per (head, kchunk) [128, 65] with ones column for the denominator."""
    BF16 = mybir.dt.bfloat16
    v_sb = P["v_p"].tile([128, HPC * NKC * 65], BF16, tag="v")
    nc.vector.memset(
        v_sb[:].rearrange("p (hj x) -> p hj x", x=65)[:, :, 64:65], 1.0
    )
    P["v_sb"] = v_sb


def _v_chunks(nc, P, js, mybir):
    F32 = mybir.dt.float32
    xt_sb, wqkv_sb, v_sb = P["xt_sb"], P["wqkv_sb"], P["v_sb"]
    for j in js:
        ps = P["vp_p"].tile([128, 512], F32, tag="vp", name="vps")
        for cc in range(CC):
            nc.tensor.matmul(
                ps[:, 0:256],
                xt_sb[:, cc * T + j * 128 : cc * T + (j + 1) * 128],
                wqkv_sb[:, cc * WCOL + 512 : cc * WCOL + 768],
                start=(cc == 0),
                stop=(cc == CC - 1),
            )
        nc.vector.tensor_copy(
            v_sb[:].rearrange("p (hj x) -> p hj x", x=65)[:, j::NKC, 0:64],
            ps[:, 0:256].rearrange("p (h d) -> p h d", d=64),
        )


def _qk_groups(nc, P, p, groups, mybir):
    """Emit q^T/k^T projection matmuls for pair p, subset of (kind, tb)."""
    F32, BF16 = mybir.dt.float32, mybir.dt.bfloat16
    xt_sb, wqkv_sb = P["xt_sb"], P["wqkv_sb"]
    for kind, tb in groups:
        dst = P["qt"][p] if kind == 0 else P["kt"][p]
        mcol = (kind * NP + p) * 128
        ps = P["mm_p"].tile([128, 1024], F32, tag="mm", name="qkps")
        for nh in range(2):
            t0 = tb * 1024 + nh * 512
            for cc in range(CC):
                nc.tensor.matmul(
                    ps[:, nh * 512 : (nh + 1) * 512],
                    wqkv_sb[:, cc * WCOL + mcol : cc * WCOL + mcol + 128],
                    xt_sb[:, cc * T + t0 : cc * T + t0 + 512],
                    start=(cc == 0),
                    stop=(cc == CC - 1),
                )
        nc.vector.tensor_copy(dst[:, tb * 1024 : (tb + 1) * 1024], ps[:])


def _attention_qg(nc, P, p, qg, mybir):
    """Scores^T + exp + AV + normalize for one head pair, one query group."""
    F32, BF16 = mybir.dt.float32, mybir.dt.bfloat16
    AFT = mybir.ActivationFunctionType
    qt, kt, v_sb, mask = P["qt"][p], P["kt"][p], P["v_sb"], P["mask"]
    outT = P[f"outT{p}"]

    njc = 4 * qg + 4          # key chunks (incl. diagonal) for this block
    avs = [
        P["av_p"].tile([65, 512], F32, tag="av", name=f"av{hh}")
        for hh in range(2)
    ]

    # pair-0 masks on DVE: the collective entry barrier can occupy the
    # gpsimd queue during the earliest (all-diagonal) query groups
    meng = nc.vector if p == 0 else nc.gpsimd

    def emit_avs(att2, jp):
        for hh in range(2):
            h = 2 * p + hh
            for dj in range(2):
                j = 2 * jp + dj
                l0 = (j - 4 * qg) * 128  # first valid query col (diag)
                lo = max(l0, 0)
                if l0 >= 0:  # diagonal chunk: triangular 0/1 mask
                    meng.tensor_mul(
                        att2[hh][:, dj * 512 + l0 : dj * 512 + l0 + 128],
                        att2[hh][:, dj * 512 + l0 : dj * 512 + l0 + 128],
                        mask[:],
                    )
                nc.tensor.matmul(
                    avs[hh][:, lo:],
                    v_sb[:, (h * NKC + j) * 65 : (h * NKC + j) * 65 + 65],
                    att2[hh][:, dj * 512 + lo : (dj + 1) * 512],
                    start=(j == 0),
                    stop=(j == njc - 1),
                )

    pend = None  # 1-deep software pipeline: scores(jp+1) before AV(jp)
    for jp in range(njc // 2):
        att2 = [None, None]
        for hh in range(2):
            ps = P["mm_p"].tile([128, 1024], F32, tag="mm", name="scps")
            for dj in range(2):
                j = 2 * jp + dj
                lo = max((j - 4 * qg) * 128, 0)  # skip sub-causal columns
                nc.tensor.matmul(
                    ps[:, dj * 512 + lo : (dj + 1) * 512],
                    kt[hh * 64 : (hh + 1) * 64, j * 128 : (j + 1) * 128],
                    qt[hh * 64 : (hh + 1) * 64, qg * 512 + lo : (qg + 1) * 512],
                    start=True,
                    stop=True,
                    tile_position=(hh * 64, 0),
                )
            a2 = P["att_p"].tile([128, 1024], BF16, tag="att", name="a2")
            nc.scalar.activation(a2[:], ps[:], AFT.Exp)
            att2[hh] = a2
        if pend is not None:
            emit_avs(*pend)
        pend = (att2, jp)
    emit_avs(*pend)
    # normalize: recip(den row) -> PE broadcast -> copy + multiply
    for hh in range(2):
        recb = P["sm_p"].tile([128, 512], F32, tag="recb")
        nc.vector.reciprocal(recb[64:65, :], avs[hh][64:65, :])
        bc = P["bc_p"].tile([128, 512], F32, tag="bc")
        nc.tensor.matmul(
            bc[0:64, :], P["ones"][64:65, :], recb[64:65, :],
            start=True, stop=True, tile_position=(64, 0),
        )
        dst = outT[hh][:, qg * 512 : (qg + 1) * 512]
        nc.vector.tensor_copy(dst, avs[hh][0:64, :])
        nc.vector.tensor_mul(dst, dst, bc[0:64, :])
        # stream this block to the pair's DRAM bounce right away
        bnc = P[f"bounce{p}"].rearrange("(q f) -> q f", q=128)
        nc.sync.dma_start(
            bnc[hh * 64 : (hh + 1) * 64, qg * 512 : (qg + 1) * 512], dst
        )


def _ag_pair(nc, P, p, mybir):
    """AllGather the pair's bounced out^T across the 4 head-group ranks."""
    groups = [[0, 1, 2, 3], [4, 5, 6, 7]]
    nc.gpsimd.collective_compute(
        "AllGather", mybir.AluOpType.bypass, replica_groups=groups,
        ins=[P[f"bounce{p}"].opt()], outs=[P[f"gath{p}"].opt()],
    )


def _gat_reads(nc, P, p, mybir):
    """Pull my column block of pair p's gathered out^T into SBUF."""
    CH = 128 * T
    for r in range(G):
        cc = 2 * r + p
        sec = P[f"gath{p}"][r * CH : (r + 1) * CH].rearrange("(q f) -> q f", q=128)
        for c in range(G):
            nc.sync.dma_start(
                P["gat"][:, cc * 512 : (cc + 1) * 512],
                sec[:, c * 512 : (c + 1) * 512],
                cond=P["condss"][0][c],
            )


def _wo_pass(nc, P, p, mybir):
    """Half of the Wo contraction (pair p's 4 chunks) for all 8 y blocks."""
    F32 = mybir.dt.float32
    gat, wo_sb, y_sb = P["gat"], P["wo_sb"], P["y_sb"]
    for t in range(4):
        for hf in range(2):
            ps = P["mm_p"].tile([128, 1024], F32, tag="mm", name="wops")
            for i, r in enumerate(range(G)):
                cc = 2 * r + p
                nc.tensor.matmul(
                    ps[:, 0:512],
                    gat[:, cc * 512 + t * 128 : cc * 512 + (t + 1) * 128],
                    wo_sb[:, cc * C + hf * 512 : cc * C + (hf + 1) * 512],
                    start=(i == 0),
                    stop=(i == G - 1),
                )
            blk = y_sb[:, (t * 2 + hf) * 512 : (t * 2 + hf + 1) * 512]
            if p == 0:
                nc.vector.tensor_copy(blk, ps[:, 0:512])
            else:
                nc.vector.tensor_add(blk, blk, ps[:, 0:512])
                nc.sync.dma_start(
                    P["out_ext"][
                        t * 128 : (t + 1) * 128, hf * 512 : (hf + 1) * 512
                    ],
                    blk,
                )


def _body(nc, P, mybir):
    BF16 = mybir.dt.bfloat16
    _load_phase(nc, P, mybir)

    for p in range(NP):
        qt_t = P["qk_p"].tile([128, T], BF16, tag=f"qt{p}", name=f"qt{p}")
        kt_t = P["qk_p"].tile([128, T], BF16, tag=f"kt{p}", name=f"kt{p}")
        P.setdefault("qt", []).append(qt_t)
        P.setdefault("kt", []).append(kt_t)
        outT = [
            P["outT_p"].tile([64, T], BF16, tag=f"outT{p}{hh}", name=f"outT{p}{hh}")
            for hh in range(2)
        ]
        P[f"outT{p}"] = outT

    # wo-phase shared state: rank condition registers, gather dest, y accum
    F32 = mybir.dt.float32
    BF16m = mybir.dt.bfloat16
    greg = nc.sync.alloc_register("gs")
    nc.sync.reg_load(greg, P["pidmod_ext"][0:1, 0:1])
    g = nc.sync.snap(greg, donate=True, min_val=0, max_val=G - 1)
    P["condss"] = [[g == c for c in range(G)]]
    P["gat"] = P["gat_p"].tile([128, CC * 512], BF16m, tag="gat", name="gat")
    P["y_sb"] = P["y_p"].tile([128, 8 * 512], F32, tag="y", name="y_sb")
    CH = 128 * T
    for p in range(NP):
        P[f"bounce{p}"] = P["dram_p"].tile(
            [CH], BF16m, tag=f"bounce{p}", name=f"bounce{p}"
        )
        P[f"gath{p}"] = P["dram_p"].tile(
            [G * CH], BF16m, tag=f"gath{p}", name=f"gath{p}"
        )

    # start attention p0 as early as possible: only the qk/v pieces each
    # query group needs are emitted ahead of it; the rest fill PE gaps
    _v_init(nc, P, mybir)
    _qk_groups(nc, P, 0, [(1, 0), (0, 0)], mybir)
    _v_chunks(nc, P, [0, 1, 2, 3], mybir)
    _attention_qg(nc, P, 0, 0, mybir)
    _v_chunks(nc, P, [4, 5, 6, 7], mybir)
    _qk_groups(nc, P, 0, [(1, 1)], mybir)
    _attention_qg(nc, P, 0, 1, mybir)
    _v_chunks(nc, P, [8, 9, 10, 11], mybir)
    _qk_groups(nc, P, 0, [(0, 1)], mybir)
    _attention_qg(nc, P, 0, 2, mybir)
    _v_chunks(nc, P, [12, 13, 14, 15], mybir)
    _qk_groups(nc, P, 1, [(1, 0)], mybir)
    _attention_qg(nc, P, 0, 3, mybir)
    _qk_groups(nc, P, 1, [(1, 1), (0, 0), (0, 1)], mybir)
    _load_wo(nc, P, mybir)
    _ag_pair(nc, P, 0, mybir)
    _gat_reads(nc, P, 0, mybir)
    for qg in range(NQG):
        _attention_qg(nc, P, 1, qg, mybir)
    _ag_pair(nc, P, 1, mybir)
    # pair-0 half of Wo runs while the second AllGather is in flight
    _wo_pass(nc, P, 0, mybir)
    _gat_reads(nc, P, 1, mybir)
    _wo_pass(nc, P, 1, mybir)


def _build():
    import concourse.mybir as mybir
    import concourse.tile as tile
    from concourse import bacc

    F32, BF16 = mybir.dt.float32, mybir.dt.bfloat16

    nc = bacc.Bacc("TRN2", target_bir_lowering=False, debug=False, num_devices=8)
    P = {
        "xt_ext": nc.declare_dram_parameter("xt", [C, T], BF16, isOutput=False),
        "wqkv_ext": nc.declare_dram_parameter("wqkv", [C, WCOL], BF16, isOutput=False),
        "wo_ext": nc.declare_dram_parameter("wo", [C, C], BF16, isOutput=False),
        "mask_ext": nc.declare_dram_parameter("mask", [128, 128], BF16, isOutput=False),
        "pidmod_ext": nc.declare_dram_parameter(
            "pidmod", [1, 1], mybir.dt.uint32, isOutput=False
        ),
        "out_ext": nc.declare_dram_parameter("out", [TOWN, C], F32, isOutput=True),
    }

    with tile.TileContext(nc) as tc:
        with (
            tc.tile_pool(name="const", bufs=1) as const_p,
            tc.tile_pool(name="w", bufs=1) as w_p,
            tc.tile_pool(name="x", bufs=1) as x_p,
            tc.tile_pool(name="qk", bufs=1) as qk_p,
            tc.tile_pool(name="v", bufs=1) as v_p,
            tc.tile_pool(name="att", bufs=4) as att_p,
            tc.tile_pool(name="outT", bufs=1) as outT_p,
            tc.tile_pool(name="gat", bufs=1) as gat_p,
            tc.tile_pool(name="y", bufs=2) as y_p,
            tc.tile_pool(name="sm", bufs=2) as sm_p,
            tc.tile_pool(name="mm", bufs=2, space="PSUM") as mm_p,
            tc.tile_pool(name="av", bufs=2, space="PSUM") as av_p,
            tc.tile_pool(name="bc", bufs=1, space="PSUM") as bc_p,
            tc.tile_pool(name="vp", bufs=1, space="PSUM") as vp_p,
            tc.tile_pool(name="dram", bufs=1, space="DRAM") as dram_p,
        ):
            P.update(
                const_p=const_p, w_p=w_p, x_p=x_p, qk_p=qk_p, v_p=v_p,
                att_p=att_p, outT_p=outT_p, gat_p=gat_p, y_p=y_p, sm_p=sm_p,
                mm_p=mm_p, av_p=av_p, bc_p=bc_p, vp_p=vp_p, dram_p=dram_p,
            )
            _body(nc, P, mybir)

    nc.finalize()
    return nc


def kernel(x, Wqkv, bqkv, Wo, bo):
    global _cached_nc, last_result
    import ml_dtypes
    from concourse.bass_utils import run_bass_kernel_spmd

    if _cached_nc is None:
        _cached_nc = _build()
    nc = _cached_nc

    bf16 = ml_dtypes.bfloat16
    x = np.asarray(x, dtype=np.float32)
    Wqkv = np.asarray(Wqkv, dtype=np.float32)
    wo_b = np.ascontiguousarray(np.asarray(Wo, dtype=np.float32).astype(bf16))

    # lower-triangle 0/1 mask for diagonal blocks: partition = key, free = query
    tri = (np.arange(128)[:, None] <= np.arange(128)[None, :]).astype(bf16)
    tri = np.ascontiguousarray(tri)

    in_maps = []
    for core in range(8):
        b, g = divmod(core, G)
        xt = np.ascontiguousarray(x[b].T.astype(bf16))
        c0 = g * HPC * D
        wq = Wqkv[:, c0 : c0 + HPC * D] * SCALE
        wk = Wqkv[:, C + c0 : C + c0 + HPC * D]
        wv = Wqkv[:, 2 * C + c0 : 2 * C + c0 + HPC * D]
        wqkv = np.ascontiguousarray(
            np.concatenate([wq, wk, wv], axis=1).astype(bf16)
        )
        in_maps.append(
            {
                "xt": xt,
                "wqkv": wqkv,
                "wo": wo_b,
                "mask": tri,
                "pidmod": np.array([[g]], dtype=np.uint32),
            }
        )

    last_result = run_bass_kernel_spmd(nc, in_maps, core_ids=list(range(8)))

    y = np.empty((B, T, C), dtype=np.float32)
    for core in range(8):
        b, g = divmod(core, G)
        y[b, g * TOWN : (g + 1) * TOWN, :] = last_result.results[core]["out"]
    return y



# revision 6
# speedup vs baseline: 1.3970x; 1.3970x over previous
"""Distributed causal multi-head attention (Bass/Tile, 8 TRN2 NeuronCores).

Sharding: core c owns heads (2c, 2c+1) of BOTH batches, and owns output
rows [c*256, (c+1)*256) of each batch.  QKV for its 2 heads is computed
from the full x (both batches) locally -- no K/V collective.  After each
batch's attention, one 8-rank AllToAll redistributes UNNORMALIZED
attention outputs plus the softmax denominator row (ones-column trick)
to the query-row owners; each core then normalizes (batched fast
reciprocal + PE broadcast) and applies the full Wo to its rows.

Per core, per batch b:
  q^T, k^T = (x_b @ Wq/Wk)^T  [128, 2048]  (partitions = 2 heads x 64 dims)
  v        =  x_b @ Wv        [128, 65] per (head, kchunk), ones col appended
  scores^T = k^T.T @ q^T      head-interleaved pairs run concurrently on PE
  softmax: full-width exp (scalar engine), causal mask as 0/1 post-mult,
  AV matmul with ones column -> unnormalized out^T + denominator row,
  AllToAll (bf16) across all 8 cores, normalize after gather, y = out^T.T @ Wo.

All projection / v / Wo matmul work is chopped into ~1-2us units and
drained into the attention loop between exp-paced iterations so the PE
stays dense (HAM clock-gate stays warm).
"""

import numpy as np

B, T, C, H = 2, 2048, 1024, 16
D = C // H            # 64
NQG = T // 512        # 4 query groups of 512
NKC = T // 128        # 16 key chunks of 128
CC = C // 128         # 8 contraction chunks
WCOL = 3 * 2 * D      # 384 packed qkv columns per core (2 heads)
SCALE = 1.0 / 32.0    # 1/sqrt(C), folded into Wq on host
OWN = 256             # query rows owned per (core, batch)
SH = 130 * OWN        # AllToAll shard elems: 2 heads x (64 d + den) x 256 q
HOFF = 65 * OWN       # head hh=1 offset inside a shard

_cached_nc = None
last_result = None


def _loads(nc, P, mybir):
    from concourse.bass import ts

    F32, BF16 = mybir.dt.float32, mybir.dt.bfloat16
    AFT = mybir.ActivationFunctionType

    # small consts first on sync
    mask = P["const_p"].tile([128, 128], BF16, tag="mask")
    nc.sync.dma_start(mask[:], P["mask_ext"][:])
    P["mask"] = mask
    sel = P["const_p"].tile([16, 1024], BF16, tag="sel", name="sel")
    nc.sync.dma_start(sel[:], P["sel_ext"][:])
    P["sel"] = sel

    # x^T for both batches, column halves (batch0 cols 0:1024 first) on sync
    xt_sb = P["x_p"].tile([128, 2 * CC * T], BF16, tag="xt")
    P["xt_sb"] = xt_sb
    order = [(0, 0), (0, 1), (1, 0), (1, 1)]
    for b, th in order:
        for cc in range(CC):
            nc.sync.dma_start(
                xt_sb[:, (b * CC + cc) * T + th * 1024 :
                      (b * CC + cc) * T + (th + 1) * 1024],
                P["xt_ext"][b * C + cc * 128 : b * C + (cc + 1) * 128,
                            th * 1024 : (th + 1) * 1024],
            )

    # packed qkv weights on scalar (before any exp is emitted)
    wqkv_sb = P["w_p"].tile([128, CC * WCOL], BF16, tag="wqkv")
    for cc in range(CC):
        nc.scalar.dma_start(
            wqkv_sb[:, cc * WCOL : (cc + 1) * WCOL], P["wqkv_ext"][ts(cc, 128), :]
        )
    P["wqkv_sb"] = wqkv_sb

    # warm the ACT exp table while DMAs stream
    scr = P["const_p"].tile([128, 32], F32, tag="scr", name="scr")
    nc.vector.memset(scr[:, 0:16], 0.0)
    nc.scalar.activation(scr[:, 16:32], scr[:, 0:16], AFT.Exp)

    # Wo on sync behind x^T (needed only ~60us in)
    wo_sb = P["w_p"].tile([128, CC * C], BF16, tag="wo")
    for cc in range(CC):
        nc.sync.dma_start(wo_sb[:, cc * C : (cc + 1) * C], P["wo_ext"][ts(cc, 128), :])
    P["wo_sb"] = wo_sb

    # ones column of v (softmax denominator accumulator)
    for b in range(2):
        v_sb = P["v_p"].tile([128, 2 * NKC * 65], BF16, tag=f"v{b}", name=f"v{b}")
        nc.vector.memset(
            v_sb[:].rearrange("p (hj x) -> p hj x", x=65)[:, :, 64:65], 1.0
        )
        P[f"v{b}"] = v_sb


def _v_unit(nc, P, b, js, mybir):
    """v rows for batch b, key chunks js: [128, 65] per (head, kchunk)."""
    F32 = mybir.dt.float32
    xt_sb, wqkv_sb, v_sb = P["xt_sb"], P["wqkv_sb"], P[f"v{b}"]
    for j in js:
        ps = P["aux_p"].tile([128, 512], F32, tag="aux", name="vps")
        for cc in range(CC):
            nc.tensor.matmul(
                ps[:, 0:128],
                xt_sb[:, (b * CC + cc) * T + j * 128 : (b * CC + cc) * T + (j + 1) * 128],
                wqkv_sb[:, cc * WCOL + 256 : cc * WCOL + 384],
                start=(cc == 0),
                stop=(cc == CC - 1),
            )
        nc.vector.tensor_copy(
            v_sb[:].rearrange("p (hj x) -> p hj x", x=65)[:, j::NKC, 0:64],
            ps[:, 0:128].rearrange("p (h d) -> p h d", d=64),
        )


def _qk_unit(nc, P, b, kind, tb, nh, mybir):
    """One 512-col block of q^T or k^T for batch b (8 matmuls + 1 copy)."""
    F32 = mybir.dt.float32
    xt_sb, wqkv_sb = P["xt_sb"], P["wqkv_sb"]
    dst = P["qt"][b] if kind == 0 else P["kt"][b]
    mcol = kind * 128
    t0 = tb * 1024 + nh * 512
    ps = P["aux_p"].tile([128, 512], F32, tag="aux", name="qkps")
    for cc in range(CC):
        nc.tensor.matmul(
            ps[:],
            wqkv_sb[:, cc * WCOL + mcol : cc * WCOL + mcol + 128],
            xt_sb[:, (b * CC + cc) * T + t0 : (b * CC + cc) * T + t0 + 512],
            start=(cc == 0),
            stop=(cc == CC - 1),
        )
    nc.vector.tensor_copy(dst[:, t0 : t0 + 512], ps[:])


def _gath_reads(nc, P, b, mybir):
    """Read pair-block columns of batch b's AllToAll output (sync queue)."""
    gath = P[f"gath{b}"]
    den = P[f"den{b}"]
    gat = P[f"gat{b}"]
    gv = gath.rearrange("(s p f) -> p s f", s=8, p=130)
    # dens: row 64 (hh=0) of each shard -> den rows 0:8, row 129 -> rows 8:16
    nc.sync.dma_start(
        den[0:8, :], gv[64:65, :, :].rearrange("p s f -> (p s) f")
    )
    nc.sync.dma_start(
        den[8:16, :], gv[129:130, :, :].rearrange("p s f -> (p s) f")
    )
    # d-rows: heads 2s at partitions 0:64, heads 2s+1 at 64:128
    nc.sync.dma_start(
        gat[0:64, :].rearrange("p (s f) -> p s f", s=8), gv[0:64, :, :]
    )
    nc.sync.dma_start(
        gat[64:128, :].rearrange("p (s f) -> p s f", s=8), gv[65:129, :, :]
    )


def _recip_unit(nc, P, b, mybir):
    F32 = mybir.dt.float32
    denf = P["sm_p"].tile([16, 256], F32, tag="smf", name=f"denf{b}")
    nc.vector.tensor_copy(denf[:], P[f"den{b}"][:])
    rec = P["sm_p"].tile([16, 256], F32, tag="smf", name=f"rec{b}")
    nc.vector.reciprocal_approx_fast(out=rec[:], in_=denf[:])
    recb = P["sm_p"].tile(
        [16, 256], mybir.dt.bfloat16, tag=f"recb{b}", name=f"recb{b}"
    )
    nc.vector.tensor_copy(recb[:], rec[:])
    P[f"recb{b}"] = recb


def _bc_mult_unit(nc, P, b, mybir):
    """Normalize gat (batch b) in place: PE broadcast of 1/den + DVE mult."""
    F32 = mybir.dt.float32
    gat, sel, recb = P[f"gat{b}"], P["sel"], P[f"recb{b}"]
    for s in range(8):
        bc = P["aux_p"].tile([128, 512], F32, tag="aux", name="bc")
        nc.tensor.matmul(
            bc[:, 0:256], sel[:, s * 128 : (s + 1) * 128], recb[:],
            start=True, stop=True,
        )
        blk = gat[:, s * 256 : (s + 1) * 256]
        nc.vector.tensor_mul(blk, blk, bc[:, 0:256])


def _wo_unit(nc, P, b, thfs, mybir):
    """Wo contraction (full 1024 chan) for y blocks thfs of batch b."""
    F32 = mybir.dt.float32
    gat, wo_sb = P[f"gat{b}"], P["wo_sb"]
    for t, hf in thfs:
        ps = P["aux_p"].tile([128, 512], F32, tag="aux", name="wops")
        for s in range(8):
            nc.tensor.matmul(
                ps[:],
                gat[:, s * 256 + t * 128 : s * 256 + (t + 1) * 128],
                wo_sb[:, s * C + hf * 512 : s * C + (hf + 1) * 512],
                start=(s == 0),
                stop=(s == 7),
            )
        yb = P["y_p"].tile([128, 512], F32, tag="y", name="yb")
        nc.vector.tensor_copy(yb[:], ps[:])
        nc.sync.dma_start(
            P["out_ext"][
                b * 256 + t * 128 : b * 256 + (t + 1) * 128,
                hf * 512 : (hf + 1) * 512,
            ],
            yb[:],
        )


def _attention_qg(nc, P, b, qg, fillers, mybir):
    """Scores^T + exp + AV for batch b's two heads, one query group.

    fillers: list of zero-arg closures emitting background PE work; one is
    drained per jp iteration (after scores/exp, before the pipelined AV),
    leftovers at the end of the group.
    """
    F32, BF16 = mybir.dt.float32, mybir.dt.bfloat16
    AFT = mybir.ActivationFunctionType
    qt, kt, v_sb, mask = P["qt"][b], P["kt"][b], P[f"v{b}"], P["mask"]

    njc = 4 * qg + 4          # key chunks (incl. diagonal) for this block
    avs = [
        P["av_p"].tile([65, 512], F32, tag="av", name=f"av{hh}")
        for hh in range(2)
    ]
    # masks: vector for first half of each batch's groups (dodges collective
    # entry/trigger blockage on the gpsimd queue), gpsimd for the rest
    meng = nc.vector if qg < 2 else nc.gpsimd

    def emit_avs(att2, jp):
        for hh in range(2):
            for dj in range(2):
                j = 2 * jp + dj
                lo = max((j - 4 * qg) * 128, 0)
                nc.tensor.matmul(
                    avs[hh][:, lo:],
                    v_sb[:, (hh * NKC + j) * 65 : (hh * NKC + j) * 65 + 65],
                    att2[hh][:, dj * 512 + lo : (dj + 1) * 512],
                    start=(j == 0),
                    stop=(j == njc - 1),
                )

    pend = None  # 1-deep software pipeline: scores(jp+1) before AV(jp)
    for jp in range(njc // 2):
        # interleave the two heads' score matmuls so the (0,0)/(64,0) PE
        # tiles run concurrently
        ps2 = [
            P["mm_p"].tile([128, 1024], F32, tag="mm", name=f"scps{hh}")
            for hh in range(2)
        ]
        for dj in range(2):
            j = 2 * jp + dj
            lo = max((j - 4 * qg) * 128, 0)  # skip sub-causal columns
            for hh in range(2):
                nc.tensor.matmul(
                    ps2[hh][:, dj * 512 + lo : (dj + 1) * 512],
                    kt[hh * 64 : (hh + 1) * 64, j * 128 : (j + 1) * 128],
                    qt[hh * 64 : (hh + 1) * 64, qg * 512 + lo : (qg + 1) * 512],
                    start=True,
                    stop=True,
                    tile_position=(hh * 64, 0),
                )
        st = 256 if jp == njc // 2 - 1 else 0  # last jp: cols<256 sub-causal
        att2 = []
        for hh in range(2):
            a2 = P["att_p"].tile([128, 1024], BF16, tag="att", name="a2")
            nc.scalar.activation(a2[:, st:], ps2[hh][:, st:], AFT.Exp)
            att2.append(a2)
        for dj in range(2):
            j = 2 * jp + dj
            l0 = (j - 4 * qg) * 128
            if l0 >= 0:  # diagonal chunk: triangular 0/1 mask
                for hh in range(2):
                    meng.tensor_mul(
                        att2[hh][:, dj * 512 + l0 : dj * 512 + l0 + 128],
                        att2[hh][:, dj * 512 + l0 : dj * 512 + l0 + 128],
                        mask[:],
                    )
        if fillers:
            fillers.pop(0)()
        if pend is not None:
            emit_avs(*pend)
        pend = (att2, jp)
    for f in fillers:
        f()
    fillers.clear()
    emit_avs(*pend)

    # evacuate unnormalized out^T + den row straight to the bounce shards
    # (the 512-query group spans two owners: shards 2qg and 2qg+1)
    bnc = P[f"bounce{b}"]
    for hh in range(2):
        ob = P["ob_p"].tile([65, 512], BF16, tag="ob", name="ob")
        nc.vector.tensor_copy(ob[:], avs[hh][:])
        for half in range(2):
            sh = 2 * qg + half
            nc.sync.dma_start(
                bnc[sh * SH + hh * HOFF : sh * SH + hh * HOFF + HOFF].rearrange(
                    "(q f) -> q f", q=65
                ),
                ob[:, half * 256 : (half + 1) * 256],
            )


def _a2a(nc, P, b, mybir):
    """AllToAll batch b's bounced shards across all 8 cores."""
    groups = [[0, 1, 2, 3, 4, 5, 6, 7]]
    nc.gpsimd.collective_compute(
        "AllToAll", mybir.AluOpType.bypass, replica_groups=groups,
        ins=[P[f"bounce{b}"].opt()], outs=[P[f"gath{b}"].opt()],
    )


def _body(nc, P, mybir):
    F32, BF16 = mybir.dt.float32, mybir.dt.bfloat16
    _loads(nc, P, mybir)

    for b in range(2):
        qt_t = P["qk_p"].tile([128, T], BF16, tag=f"qt{b}", name=f"qt{b}")
        kt_t = P["qk_p"].tile([128, T], BF16, tag=f"kt{b}", name=f"kt{b}")
        P.setdefault("qt", []).append(qt_t)
        P.setdefault("kt", []).append(kt_t)
        P[f"gat{b}"] = P["gat_p"].tile(
            [128, 8 * OWN], BF16, tag=f"gat{b}", name=f"gat{b}"
        )
        P[f"den{b}"] = P["sm_p"].tile(
            [16, 256], BF16, tag=f"den{b}", name=f"den{b}"
        )
        P[f"bounce{b}"] = P["dram_p"].tile(
            [8 * SH], BF16, tag=f"bounce{b}", name=f"bounce{b}"
        )
        P[f"gath{b}"] = P["dram_p"].tile(
            [8 * SH], BF16, tag=f"gath{b}", name=f"gath{b}"
        )

    U = lambda *a: (lambda: _qk_unit(nc, P, *a, mybir))
    V = lambda b, *js: (lambda: _v_unit(nc, P, b, js, mybir))
    WO = lambda b, *thfs: (lambda: _wo_unit(nc, P, b, thfs, mybir))

    # lead-in: just enough q^T/k^T/v for batch-0 qg0
    _qk_unit(nc, P, 0, 1, 0, 0, mybir)
    _qk_unit(nc, P, 0, 0, 0, 0, mybir)
    _v_unit(nc, P, 0, [0, 1], mybir)

    _attention_qg(nc, P, 0, 0, [V(0, 2, 3), U(0, 1, 0, 1), U(0, 0, 0, 1)], mybir)
    _attention_qg(
        nc, P, 0, 1, [V(0, 4, 5), V(0, 6, 7), U(0, 1, 1, 0), U(0, 0, 1, 0)], mybir
    )
    _attention_qg(
        nc, P, 0, 2,
        [V(0, 8, 9), V(0, 10, 11), U(0, 1, 1, 1), U(0, 0, 1, 1),
         U(1, 1, 0, 0), U(1, 0, 0, 0)],
        mybir,
    )
    _attention_qg(
        nc, P, 0, 3,
        [V(0, 12, 13), V(0, 14, 15), V(1, 0, 1), V(1, 2, 3),
         U(1, 1, 0, 1), U(1, 0, 0, 1), U(1, 1, 1, 0), U(1, 0, 1, 0)],
        mybir,
    )
    _a2a(nc, P, 0, mybir)

    _attention_qg(nc, P, 1, 0, [V(1, 4, 5), V(1, 6, 7)], mybir)
    _attention_qg(
        nc, P, 1, 1,
        [V(1, 8, 9), V(1, 10, 11), U(1, 1, 1, 1), U(1, 0, 1, 1),
         lambda: _gath_reads(nc, P, 0, mybir),
         lambda: _recip_unit(nc, P, 0, mybir)],
        mybir,
    )
    _attention_qg(
        nc, P, 1, 2,
        [V(1, 12, 13), V(1, 14, 15),
         lambda: _bc_mult_unit(nc, P, 0, mybir),
         WO(0, (0, 0)), WO(0, (0, 1))],
        mybir,
    )
    _attention_qg(nc, P, 1, 3, [WO(0, (1, 0)), WO(0, (1, 1))], mybir)
    _a2a(nc, P, 1, mybir)

    _gath_reads(nc, P, 1, mybir)
    _recip_unit(nc, P, 1, mybir)
    _bc_mult_unit(nc, P, 1, mybir)
    _wo_unit(nc, P, 1, [(0, 0), (0, 1)], mybir)
    _wo_unit(nc, P, 1, [(1, 0), (1, 1)], mybir)


def _build():
    import concourse.mybir as mybir
    import concourse.tile as tile
    from concourse import bacc

    F32, BF16 = mybir.dt.float32, mybir.dt.bfloat16

    nc = bacc.Bacc("TRN2", target_bir_lowering=False, debug=False, num_devices=8)
    P = {
        "xt_ext": nc.declare_dram_parameter("xt", [2 * C, T], BF16, isOutput=False),
        "wqkv_ext": nc.declare_dram_parameter("wqkv", [C, WCOL], BF16, isOutput=False),
        "wo_ext": nc.declare_dram_parameter("wo", [C, C], BF16, isOutput=False),
        "mask_ext": nc.declare_dram_parameter("mask", [128, 128], BF16, isOutput=False),
        "sel_ext": nc.declare_dram_parameter("sel", [16, 1024], BF16, isOutput=False),
        "out_ext": nc.declare_dram_parameter("out", [2 * OWN, C], F32, isOutput=True),
    }

    with tile.TileContext(nc) as tc:
        with (
            tc.tile_pool(name="const", bufs=1) as const_p,
            tc.tile_pool(name="w", bufs=1) as w_p,
            tc.tile_pool(name="x", bufs=1) as x_p,
            tc.tile_pool(name="qk", bufs=1) as qk_p,
            tc.tile_pool(name="v", bufs=1) as v_p,
            tc.tile_pool(name="att", bufs=4) as att_p,
            tc.tile_pool(name="ob", bufs=4) as ob_p,
            tc.tile_pool(name="gat", bufs=1) as gat_p,
            tc.tile_pool(name="y", bufs=2) as y_p,
            tc.tile_pool(name="sm", bufs=2) as sm_p,
            tc.tile_pool(name="mm", bufs=2, space="PSUM") as mm_p,
            tc.tile_pool(name="av", bufs=2, space="PSUM") as av_p,
            tc.tile_pool(name="aux", bufs=2, space="PSUM") as aux_p,
            tc.tile_pool(name="dram", bufs=1, space="DRAM") as dram_p,
        ):
            P.update(
                const_p=const_p, w_p=w_p, x_p=x_p, qk_p=qk_p, v_p=v_p,
                att_p=att_p, ob_p=ob_p, gat_p=gat_p, y_p=y_p, sm_p=sm_p,
                mm_p=mm_p, av_p=av_p, aux_p=aux_p, dram_p=dram_p,
            )
            _body(nc, P, mybir)

    nc.finalize()
    return nc


def kernel(x, Wqkv, bqkv, Wo, bo):
    global _cached_nc, last_result
    import ml_dtypes
    from concourse.bass_utils import run_bass_kernel_spmd

    if _cached_nc is None:
        _cached_nc = _build()
    nc = _cached_nc

    bf16 = ml_dtypes.bfloat16
    x = np.asarray(x, dtype=np.float32)
    Wqkv = np.asarray(Wqkv, dtype=np.float32)
    wo_b = np.ascontiguousarray(np.asarray(Wo, dtype=np.float32).astype(bf16))

    # x^T for both batches stacked: rows [b*C + c]
    xt = np.ascontiguousarray(
        np.concatenate([x[0].T, x[1].T], axis=0).astype(bf16)
    )

    # lower-triangle 0/1 mask for diagonal blocks: partition = key, free = query
    tri = (np.arange(128)[:, None] <= np.arange(128)[None, :]).astype(bf16)
    tri = np.ascontiguousarray(tri)

    # selection matrix for the 1/den PE broadcast: chunk s rows 0:64 get
    # head 2s's den (sel row s), rows 64:128 get head 2s+1's (row 8+s)
    sel = np.zeros((16, 1024), dtype=bf16)
    for s in range(8):
        sel[s, s * 128 : s * 128 + 64] = 1
        sel[8 + s, s * 128 + 64 : s * 128 + 128] = 1
    sel = np.ascontiguousarray(sel)

    in_maps = []
    for core in range(8):
        c0 = core * 2 * D
        wq = Wqkv[:, c0 : c0 + 128] * SCALE
        wk = Wqkv[:, C + c0 : C + c0 + 128]
        wv = Wqkv[:, 2 * C + c0 : 2 * C + c0 + 128]
        wqkv = np.ascontiguousarray(
            np.concatenate([wq, wk, wv], axis=1).astype(bf16)
        )
        in_maps.append(
            {"xt": xt, "wqkv": wqkv, "wo": wo_b, "mask": tri, "sel": sel}
        )

    last_result = run_bass_kernel_spmd(nc, in_maps, core_ids=list(range(8)))

    y = np.empty((B, T, C), dtype=np.float32)
    for core in range(8):
        r = last_result.results[core]["out"]
        y[0, core * OWN : (core + 1) * OWN, :] = r[0:OWN]
        y[1, core * OWN : (core + 1) * OWN, :] = r[OWN : 2 * OWN]
    return y


# revision 7
# speedup vs baseline: 1.4160x; 1.0136x over previous
"""Distributed causal multi-head attention (Bass/Tile, 8 TRN2 NeuronCores).

Sharding: core c owns heads (2c, 2c+1) of BOTH batches, and owns output
rows [c*256, (c+1)*256) of each batch.  QKV for its 2 heads is computed
from the full x (both batches) locally -- no K/V collective.  After each
batch's attention, one 8-rank AllToAll redistributes UNNORMALIZED
attention outputs plus the softmax denominator row (ones-column trick)
to the query-row owners; each core then normalizes (batched fast
reciprocal + PE broadcast) and applies the full Wo to its rows.

Per core, per batch b:
  q^T, k^T = (x_b @ Wq/Wk)^T  [128, 2048]  (partitions = 2 heads x 64 dims)
  v        =  x_b @ Wv        [128, 65] per (head, kchunk), ones col appended
  scores^T = k^T.T @ q^T      head-interleaved pairs run concurrently on PE
  softmax: full-width exp (scalar engine), causal mask as 0/1 post-mult,
  AV matmul with ones column -> unnormalized out^T + denominator row,
  AllToAll (bf16) across all 8 cores, normalize after gather, y = out^T.T @ Wo.

All projection / v / Wo matmul work is chopped into ~1-2us units and
drained into the attention loop between exp-paced iterations so the PE
stays dense (HAM clock-gate stays warm).
"""

import numpy as np

B, T, C, H = 2, 2048, 1024, 16
D = C // H            # 64
NQG = T // 512        # 4 query groups of 512
NKC = T // 128        # 16 key chunks of 128
CC = C // 128         # 8 contraction chunks
WCOL = 3 * 2 * D      # 384 packed qkv columns per core (2 heads)
SCALE = 1.0 / 32.0    # 1/sqrt(C), folded into Wq on host
OWN = 256             # query rows owned per (core, batch)
SH = 130 * OWN        # AllToAll shard elems: 2 heads x (64 d + den) x 256 q
HOFF = 65 * OWN       # head hh=1 offset inside a shard

_cached_nc = None
last_result = None


def _loads(nc, P, mybir):
    from concourse.bass import ts

    F32, BF16 = mybir.dt.float32, mybir.dt.bfloat16
    AFT = mybir.ActivationFunctionType

    # small consts first on sync
    mask = P["const_p"].tile([128, 128], BF16, tag="mask")
    nc.sync.dma_start(mask[:], P["mask_ext"][:])
    P["mask"] = mask
    sel = P["const_p"].tile([16, 1024], BF16, tag="sel", name="sel")
    nc.sync.dma_start(sel[:], P["sel_ext"][:])
    P["sel"] = sel

    # x^T for both batches, column halves (batch0 cols 0:1024 first) on sync
    xt_sb = P["x_p"].tile([128, 2 * CC * T], BF16, tag="xt")
    P["xt_sb"] = xt_sb
    order = [(0, 0), (0, 1), (1, 0), (1, 1)]
    for b, th in order:
        for cc in range(CC):
            nc.sync.dma_start(
                xt_sb[:, (b * CC + cc) * T + th * 1024 :
                      (b * CC + cc) * T + (th + 1) * 1024],
                P["xt_ext"][b * C + cc * 128 : b * C + (cc + 1) * 128,
                            th * 1024 : (th + 1) * 1024],
            )

    # packed qkv weights on scalar (before any exp is emitted)
    wqkv_sb = P["w_p"].tile([128, CC * WCOL], BF16, tag="wqkv")
    for cc in range(CC):
        nc.scalar.dma_start(
            wqkv_sb[:, cc * WCOL : (cc + 1) * WCOL], P["wqkv_ext"][ts(cc, 128), :]
        )
    P["wqkv_sb"] = wqkv_sb

    # warm the ACT exp table while DMAs stream
    scr = P["const_p"].tile([128, 32], F32, tag="scr", name="scr")
    nc.vector.memset(scr[:, 0:16], 0.0)
    nc.scalar.activation(scr[:, 16:32], scr[:, 0:16], AFT.Exp)

    # Wo on sync behind x^T (needed only ~60us in)
    wo_sb = P["w_p"].tile([128, CC * C], BF16, tag="wo")
    for cc in range(CC):
        nc.sync.dma_start(wo_sb[:, cc * C : (cc + 1) * C], P["wo_ext"][ts(cc, 128), :])
    P["wo_sb"] = wo_sb

    # ones column of v (softmax denominator accumulator)
    for b in range(2):
        v_sb = P["v_p"].tile([128, 2 * NKC * 65], BF16, tag=f"v{b}", name=f"v{b}")
        nc.vector.memset(
            v_sb[:].rearrange("p (hj x) -> p hj x", x=65)[:, :, 64:65], 1.0
        )
        P[f"v{b}"] = v_sb


def _v_unit(nc, P, b, js, mybir):
    """v rows for batch b, key chunks js: [128, 65] per (head, kchunk)."""
    F32 = mybir.dt.float32
    xt_sb, wqkv_sb, v_sb = P["xt_sb"], P["wqkv_sb"], P[f"v{b}"]
    for j in js:
        ps = P["aux_p"].tile([128, 512], F32, tag="aux", name="vps")
        for cc in range(CC):
            nc.tensor.matmul(
                ps[:, 0:128],
                xt_sb[:, (b * CC + cc) * T + j * 128 : (b * CC + cc) * T + (j + 1) * 128],
                wqkv_sb[:, cc * WCOL + 256 : cc * WCOL + 384],
                start=(cc == 0),
                stop=(cc == CC - 1),
            )
        nc.vector.tensor_copy(
            v_sb[:].rearrange("p (hj x) -> p hj x", x=65)[:, j::NKC, 0:64],
            ps[:, 0:128].rearrange("p (h d) -> p h d", d=64),
        )


def _qk_unit(nc, P, b, kind, tb, nh, mybir):
    """One 512-col block of q^T or k^T for batch b (8 matmuls + 1 copy)."""
    F32 = mybir.dt.float32
    xt_sb, wqkv_sb = P["xt_sb"], P["wqkv_sb"]
    dst = P["qt"][b] if kind == 0 else P["kt"][b]
    mcol = kind * 128
    t0 = tb * 1024 + nh * 512
    ps = P["aux_p"].tile([128, 512], F32, tag="aux", name="qkps")
    for cc in range(CC):
        nc.tensor.matmul(
            ps[:],
            wqkv_sb[:, cc * WCOL + mcol : cc * WCOL + mcol + 128],
            xt_sb[:, (b * CC + cc) * T + t0 : (b * CC + cc) * T + t0 + 512],
            start=(cc == 0),
            stop=(cc == CC - 1),
        )
    nc.vector.tensor_copy(dst[:, t0 : t0 + 512], ps[:])


def _gath_reads(nc, P, b, mybir):
    """Read pair-block columns of batch b's AllToAll output (sync queue)."""
    gath = P[f"gath{b}"]
    den = P[f"den{b}"]
    gat = P[f"gat{b}"]
    gv = gath.rearrange("(s p f) -> p s f", s=8, p=130)
    # dens: row 64 (hh=0) of each shard -> den rows 0:8, row 129 -> rows 8:16
    nc.sync.dma_start(
        den[0:8, :], gv[64:65, :, :].rearrange("p s f -> (p s) f")
    )
    nc.sync.dma_start(
        den[8:16, :], gv[129:130, :, :].rearrange("p s f -> (p s) f")
    )
    # d-rows: heads 2s at partitions 0:64, heads 2s+1 at 64:128
    nc.sync.dma_start(
        gat[0:64, :].rearrange("p (s f) -> p s f", s=8), gv[0:64, :, :]
    )
    nc.sync.dma_start(
        gat[64:128, :].rearrange("p (s f) -> p s f", s=8), gv[65:129, :, :]
    )


def _recip_unit(nc, P, b, mybir):
    F32 = mybir.dt.float32
    denf = P["sm_p"].tile([16, 256], F32, tag="smf", name=f"denf{b}")
    nc.vector.tensor_copy(denf[:], P[f"den{b}"][:])
    rec = P["sm_p"].tile([16, 256], F32, tag="smf", name=f"rec{b}")
    nc.vector.reciprocal_approx_fast(out=rec[:], in_=denf[:])
    recb = P["sm_p"].tile(
        [16, 256], mybir.dt.bfloat16, tag=f"recb{b}", name=f"recb{b}"
    )
    nc.vector.tensor_copy(recb[:], rec[:])
    P[f"recb{b}"] = recb


def _bc_mult_unit(nc, P, b, mybir):
    """Normalize gat (batch b) in place: PE broadcast of 1/den + DVE mult."""
    F32 = mybir.dt.float32
    gat, sel, recb = P[f"gat{b}"], P["sel"], P[f"recb{b}"]
    for s in range(8):
        bc = P["aux_p"].tile([128, 512], F32, tag="aux", name="bc")
        nc.tensor.matmul(
            bc[:, 0:256], sel[:, s * 128 : (s + 1) * 128], recb[:],
            start=True, stop=True,
        )
        blk = gat[:, s * 256 : (s + 1) * 256]
        nc.vector.tensor_mul(blk, blk, bc[:, 0:256])


def _wo_unit(nc, P, b, thfs, mybir):
    """Wo contraction (full 1024 chan) for y blocks thfs of batch b."""
    F32 = mybir.dt.float32
    gat, wo_sb = P[f"gat{b}"], P["wo_sb"]
    for t, hf in thfs:
        ps = P["aux_p"].tile([128, 512], F32, tag="aux", name="wops")
        for s in range(8):
            nc.tensor.matmul(
                ps[:],
                gat[:, s * 256 + t * 128 : s * 256 + (t + 1) * 128],
                wo_sb[:, s * C + hf * 512 : s * C + (hf + 1) * 512],
                start=(s == 0),
                stop=(s == 7),
            )
        yb = P["y_p"].tile([128, 512], F32, tag="y", name="yb")
        nc.vector.tensor_copy(yb[:], ps[:])
        nc.sync.dma_start(
            P["out_ext"][
                b * 256 + t * 128 : b * 256 + (t + 1) * 128,
                hf * 512 : (hf + 1) * 512,
            ],
            yb[:],
        )


def _attention_qg(nc, P, b, qg, fillers, mybir):
    """Scores^T + exp + AV for batch b's two heads, one query group.

    fillers: list of zero-arg closures emitting background PE work; one is
    drained per jp iteration (after scores/exp, before the pipelined AV),
    leftovers at the end of the group.
    """
    F32, BF16 = mybir.dt.float32, mybir.dt.bfloat16
    AFT = mybir.ActivationFunctionType
    qt, kt, v_sb, mask = P["qt"][b], P["kt"][b], P[f"v{b}"], P["mask"]

    njc = 4 * qg + 4          # key chunks (incl. diagonal) for this block
    avs = [
        P["av_p"].tile([65, 512], F32, tag="av", name=f"av{hh}")
        for hh in range(2)
    ]
    # masks: vector for first half of each batch's groups (dodges collective
    # entry/trigger blockage on the gpsimd queue), gpsimd for the rest
    meng = nc.vector if qg < 2 else nc.gpsimd

    def emit_avs(att2, jp):
        for hh in range(2):
            for dj in range(2):
                j = 2 * jp + dj
                lo = max((j - 4 * qg) * 128, 0)
                nc.tensor.matmul(
                    avs[hh][:, lo:],
                    v_sb[:, (hh * NKC + j) * 65 : (hh * NKC + j) * 65 + 65],
                    att2[hh][:, dj * 512 + lo : (dj + 1) * 512],
                    start=(j == 0),
                    stop=(j == njc - 1),
                )

    pend = None  # 1-deep software pipeline: scores(jp+1) before AV(jp)
    for jp in range(njc // 2):
        # interleave the two heads' score matmuls so the (0,0)/(64,0) PE
        # tiles run concurrently
        ps2 = [
            P["mm_p"].tile([128, 1024], F32, tag="mm", name=f"scps{hh}")
            for hh in range(2)
        ]
        for dj in range(2):
            j = 2 * jp + dj
            lo = max((j - 4 * qg) * 128, 0)  # skip sub-causal columns
            for hh in range(2):
                nc.tensor.matmul(
                    ps2[hh][:, dj * 512 + lo : (dj + 1) * 512],
                    kt[hh * 64 : (hh + 1) * 64, j * 128 : (j + 1) * 128],
                    qt[hh * 64 : (hh + 1) * 64, qg * 512 + lo : (qg + 1) * 512],
                    start=True,
                    stop=True,
                    tile_position=(hh * 64, 0),
                )
        st = 256 if jp == njc // 2 - 1 else 0  # last jp: cols<256 sub-causal
        att2 = []
        for hh in range(2):
            a2 = P["att_p"].tile([128, 1024], BF16, tag="att", name="a2")
            nc.scalar.activation(a2[:, st:], ps2[hh][:, st:], AFT.Exp)
            att2.append(a2)
        for dj in range(2):
            j = 2 * jp + dj
            l0 = (j - 4 * qg) * 128
            if l0 >= 0:  # diagonal chunk: triangular 0/1 mask
                for hh in range(2):
                    meng.tensor_mul(
                        att2[hh][:, dj * 512 + l0 : dj * 512 + l0 + 128],
                        att2[hh][:, dj * 512 + l0 : dj * 512 + l0 + 128],
                        mask[:],
                    )
        if fillers:
            fillers.pop(0)()
        if pend is not None:
            emit_avs(*pend)
        pend = (att2, jp)
    for f in fillers:
        f()
    fillers.clear()
    emit_avs(*pend)

    # evacuate unnormalized out^T + den row straight to the bounce shards
    # (the 512-query group spans two owners: shards 2qg and 2qg+1)
    bnc = P[f"bounce{b}"]
    for hh in range(2):
        ob = P["ob_p"].tile([65, 512], BF16, tag="ob", name="ob")
        nc.vector.tensor_copy(ob[:], avs[hh][:])
        for half in range(2):
            sh = 2 * qg + half
            nc.sync.dma_start(
                bnc[sh * SH + hh * HOFF : sh * SH + hh * HOFF + HOFF].rearrange(
                    "(q f) -> q f", q=65
                ),
                ob[:, half * 256 : (half + 1) * 256],
            )


def _a2a(nc, P, b, mybir):
    """AllToAll batch b's bounced shards across all 8 cores."""
    groups = [[0, 1, 2, 3, 4, 5, 6, 7]]
    nc.gpsimd.collective_compute(
        "AllToAll", mybir.AluOpType.bypass, replica_groups=groups,
        ins=[P[f"bounce{b}"].opt()], outs=[P[f"gath{b}"].opt()],
    )


def _body(nc, P, mybir):
    F32, BF16 = mybir.dt.float32, mybir.dt.bfloat16
    _loads(nc, P, mybir)

    for b in range(2):
        qt_t = P["qk_p"].tile([128, T], BF16, tag=f"qt{b}", name=f"qt{b}")
        kt_t = P["qk_p"].tile([128, T], BF16, tag=f"kt{b}", name=f"kt{b}")
        P.setdefault("qt", []).append(qt_t)
        P.setdefault("kt", []).append(kt_t)
        P[f"gat{b}"] = P["gat_p"].tile(
            [128, 8 * OWN], BF16, tag=f"gat{b}", name=f"gat{b}"
        )
        P[f"den{b}"] = P["sm_p"].tile(
            [16, 256], BF16, tag=f"den{b}", name=f"den{b}"
        )
        P[f"bounce{b}"] = P["dram_p"].tile(
            [8 * SH], BF16, tag=f"bounce{b}", name=f"bounce{b}"
        )
        P[f"gath{b}"] = P["dram_p"].tile(
            [8 * SH], BF16, tag=f"gath{b}", name=f"gath{b}"
        )

    U = lambda *a: (lambda: _qk_unit(nc, P, *a, mybir))
    V = lambda b, *js: (lambda: _v_unit(nc, P, b, js, mybir))
    WO = lambda b, *thfs: (lambda: _wo_unit(nc, P, b, thfs, mybir))

    # lead-in: just enough q^T/k^T/v for batch-0 qg0
    _qk_unit(nc, P, 0, 1, 0, 0, mybir)
    _qk_unit(nc, P, 0, 0, 0, 0, mybir)
    _v_unit(nc, P, 0, [0, 1], mybir)

    _attention_qg(nc, P, 0, 0, [V(0, 2, 3), U(0, 1, 0, 1), U(0, 0, 0, 1)], mybir)
    _attention_qg(
        nc, P, 0, 1, [V(0, 4, 5), V(0, 6, 7), U(0, 1, 1, 0), U(0, 0, 1, 0)], mybir
    )
    _attention_qg(
        nc, P, 0, 2,
        [V(0, 8, 9), V(0, 10, 11), U(0, 1, 1, 1), U(0, 0, 1, 1),
         U(1, 1, 0, 0), U(1, 0, 0, 0)],
        mybir,
    )
    _attention_qg(
        nc, P, 0, 3,
        [V(0, 12, 13), V(0, 14, 15), V(1, 0, 1), V(1, 2, 3),
         U(1, 1, 0, 1), U(1, 0, 0, 1), U(1, 1, 1, 0), U(1, 0, 1, 0)],
        mybir,
    )
    _a2a(nc, P, 0, mybir)

    _attention_qg(nc, P, 1, 0, [V(1, 4, 5), V(1, 6, 7)], mybir)
    _attention_qg(
        nc, P, 1, 1,
        [V(1, 8, 9), V(1, 10, 11), U(1, 1, 1, 1), U(1, 0, 1, 1)],
        mybir,
    )
    # A2A0-dependent work sits deep in qg2 so the DVE queue never stalls
    # on the collective (sync-queue gath reads absorb any remaining wait)
    _attention_qg(
        nc, P, 1, 2,
        [lambda: _gath_reads(nc, P, 0, mybir),
         V(1, 12, 13), V(1, 14, 15),
         lambda: _recip_unit(nc, P, 0, mybir),
         lambda: _bc_mult_unit(nc, P, 0, mybir),
         WO(0, (0, 0))],
        mybir,
    )
    _attention_qg(
        nc, P, 1, 3, [WO(0, (0, 1)), WO(0, (1, 0)), WO(0, (1, 1))], mybir
    )
    _a2a(nc, P, 1, mybir)

    _gath_reads(nc, P, 1, mybir)
    _recip_unit(nc, P, 1, mybir)
    _bc_mult_unit(nc, P, 1, mybir)
    _wo_unit(nc, P, 1, [(0, 0), (0, 1)], mybir)
    _wo_unit(nc, P, 1, [(1, 0), (1, 1)], mybir)


def _build():
    import concourse.mybir as mybir
    import concourse.tile as tile
    from concourse import bacc

    F32, BF16 = mybir.dt.float32, mybir.dt.bfloat16

    nc = bacc.Bacc("TRN2", target_bir_lowering=False, debug=False, num_devices=8)
    P = {
        "xt_ext": nc.declare_dram_parameter("xt", [2 * C, T], BF16, isOutput=False),
        "wqkv_ext": nc.declare_dram_parameter("wqkv", [C, WCOL], BF16, isOutput=False),
        "wo_ext": nc.declare_dram_parameter("wo", [C, C], BF16, isOutput=False),
        "mask_ext": nc.declare_dram_parameter("mask", [128, 128], BF16, isOutput=False),
        "sel_ext": nc.declare_dram_parameter("sel", [16, 1024], BF16, isOutput=False),
        "out_ext": nc.declare_dram_parameter("out", [2 * OWN, C], F32, isOutput=True),
    }

    with tile.TileContext(nc) as tc:
        with (
            tc.tile_pool(name="const", bufs=1) as const_p,
            tc.tile_pool(name="w", bufs=1) as w_p,
            tc.tile_pool(name="x", bufs=1) as x_p,
            tc.tile_pool(name="qk", bufs=1) as qk_p,
            tc.tile_pool(name="v", bufs=1) as v_p,
            tc.tile_pool(name="att", bufs=4) as att_p,
            tc.tile_pool(name="ob", bufs=4) as ob_p,
            tc.tile_pool(name="gat", bufs=1) as gat_p,
            tc.tile_pool(name="y", bufs=2) as y_p,
            tc.tile_pool(name="sm", bufs=2) as sm_p,
            tc.tile_pool(name="mm", bufs=2, space="PSUM") as mm_p,
            tc.tile_pool(name="av", bufs=2, space="PSUM") as av_p,
            tc.tile_pool(name="aux", bufs=2, space="PSUM") as aux_p,
            tc.tile_pool(name="dram", bufs=1, space="DRAM") as dram_p,
        ):
            P.update(
                const_p=const_p, w_p=w_p, x_p=x_p, qk_p=qk_p, v_p=v_p,
                att_p=att_p, ob_p=ob_p, gat_p=gat_p, y_p=y_p, sm_p=sm_p,
                mm_p=mm_p, av_p=av_p, aux_p=aux_p, dram_p=dram_p,
            )
            _body(nc, P, mybir)

    nc.finalize()
    return nc


def kernel(x, Wqkv, bqkv, Wo, bo):
    global _cached_nc, last_result
    import ml_dtypes
    from concourse.bass_utils import run_bass_kernel_spmd

    if _cached_nc is None:
        _cached_nc = _build()
    nc = _cached_nc

    bf16 = ml_dtypes.bfloat16
    x = np.asarray(x, dtype=np.float32)
    Wqkv = np.asarray(Wqkv, dtype=np.float32)
    wo_b = np.ascontiguousarray(np.asarray(Wo, dtype=np.float32).astype(bf16))

    # x^T for both batches stacked: rows [b*C + c]
    xt = np.ascontiguousarray(
        np.concatenate([x[0].T, x[1].T], axis=0).astype(bf16)
    )

    # lower-triangle 0/1 mask for diagonal blocks: partition = key, free = query
    tri = (np.arange(128)[:, None] <= np.arange(128)[None, :]).astype(bf16)
    tri = np.ascontiguousarray(tri)

    # selection matrix for the 1/den PE broadcast: chunk s rows 0:64 get
    # head 2s's den (sel row s), rows 64:128 get head 2s+1's (row 8+s)
    sel = np.zeros((16, 1024), dtype=bf16)
    for s in range(8):
        sel[s, s * 128 : s * 128 + 64] = 1
        sel[8 + s, s * 128 + 64 : s * 128 + 128] = 1
    sel = np.ascontiguousarray(sel)

    in_maps = []
    for core in range(8):
        c0 = core * 2 * D
        wq = Wqkv[:, c0 : c0 + 128] * SCALE
        wk = Wqkv[:, C + c0 : C + c0 + 128]
        wv = Wqkv[:, 2 * C + c0 : 2 * C + c0 + 128]
        wqkv = np.ascontiguousarray(
            np.concatenate([wq, wk, wv], axis=1).astype(bf16)
        )
        in_maps.append(
            {"xt": xt, "wqkv": wqkv, "wo": wo_b, "mask": tri, "sel": sel}
        )

    last_result = run_bass_kernel_spmd(nc, in_maps, core_ids=list(range(8)))

    y = np.empty((B, T, C), dtype=np.float32)
    for core in range(8):
        r = last_result.results[core]["out"]
        y[0, core * OWN : (core + 1) * OWN, :] = r[0:OWN]
        y[1, core * OWN : (core + 1) * OWN, :] = r[OWN : 2 * OWN]
    return y


# revision 11
# speedup vs baseline: 1.4563x; 1.0285x over previous
"""Distributed causal multi-head attention (Bass/Tile, 8 TRN2 NeuronCores).

Sharding: core c owns heads (2c, 2c+1) of BOTH batches, and owns output
rows [c*256, (c+1)*256) of each batch.  QKV for its 2 heads is computed
from the full x (both batches) locally -- no K/V collective.  After each
batch's attention, one 8-rank AllToAll redistributes UNNORMALIZED
attention outputs plus the softmax denominator row (ones-column trick)
to the query-row owners; each core then normalizes (batched fast
reciprocal + PE broadcast) and applies the full Wo to its rows.

Per core, per batch b:
  q^T, k^T = (x_b @ Wq/Wk)^T  [128, 2048]  (partitions = 2 heads x 64 dims)
  v        =  x_b @ Wv        [128, 65] per (head, kchunk), ones col appended
  scores^T = k^T.T @ q^T      head-interleaved pairs run concurrently on PE
  softmax: full-width exp (scalar engine), causal mask as 0/1 post-mult,
  AV matmul with ones column -> unnormalized out^T + denominator row,
  AllToAll (bf16) across all 8 cores, normalize after gather, y = out^T.T @ Wo.

All projection / v / Wo matmul work is chopped into ~1-2us units and
drained into the attention loop between exp-paced iterations so the PE
stays dense (HAM clock-gate stays warm).
"""

import numpy as np

B, T, C, H = 2, 2048, 1024, 16
D = C // H            # 64
NQG = T // 512        # 4 query groups of 512
NKC = T // 128        # 16 key chunks of 128
CC = C // 128         # 8 contraction chunks
WCOL = 3 * 2 * D      # 384 packed qkv columns per core (2 heads)
SCALE = 1.0 / 32.0    # 1/sqrt(C), folded into Wq on host
OWN = 256             # query rows owned per (core, batch)
SH = 130 * OWN        # AllToAll shard elems: 2 heads x (64 d + den) x 256 q
HOFF = 65 * OWN       # head hh=1 offset inside a shard

_cached_nc = None
last_result = None


def _loads(nc, P, mybir):
    from concourse.bass import ts

    F32, BF16 = mybir.dt.float32, mybir.dt.bfloat16
    AFT = mybir.ActivationFunctionType

    # small consts first on sync
    mask = P["const_p"].tile([128, 128], BF16, tag="mask")
    nc.sync.dma_start(mask[:], P["mask_ext"][:])
    P["mask"] = mask
    sel = P["const_p"].tile([16, 1024], BF16, tag="sel", name="sel")
    nc.sync.dma_start(sel[:], P["sel_ext"][:])
    P["sel"] = sel

    # x^T for both batches, column halves (batch0 cols 0:1024 first),
    # striped across the sync and gpsimd DMA queues for 2x load rate
    xt_sb = P["x_p"].tile([128, 2 * CC * T], BF16, tag="xt")
    P["xt_sb"] = xt_sb
    order = [(0, 0), (0, 1), (1, 0), (1, 1)]
    for b, th in order:
        for cc in range(CC):
            q = nc.sync if cc % 2 == 0 else nc.gpsimd
            q.dma_start(
                xt_sb[:, (b * CC + cc) * T + th * 1024 :
                      (b * CC + cc) * T + (th + 1) * 1024],
                P["xt_ext"][b * C + cc * 128 : b * C + (cc + 1) * 128,
                            th * 1024 : (th + 1) * 1024],
            )

    # packed qkv weights on scalar (before any exp is emitted)
    wqkv_sb = P["w_p"].tile([128, CC * WCOL], BF16, tag="wqkv")
    for cc in range(CC):
        nc.scalar.dma_start(
            wqkv_sb[:, cc * WCOL : (cc + 1) * WCOL], P["wqkv_ext"][ts(cc, 128), :]
        )
    P["wqkv_sb"] = wqkv_sb

    # warm the ACT exp table while DMAs stream
    scr = P["const_p"].tile([128, 32], F32, tag="scr", name="scr")
    nc.vector.memset(scr[:, 0:16], 0.0)
    nc.scalar.activation(scr[:, 16:32], scr[:, 0:16], AFT.Exp)

    # Wo on sync behind x^T (needed only ~60us in)
    wo_sb = P["w_p"].tile([128, CC * C], BF16, tag="wo")
    for cc in range(CC):
        nc.sync.dma_start(wo_sb[:, cc * C : (cc + 1) * C], P["wo_ext"][ts(cc, 128), :])
    P["wo_sb"] = wo_sb

    # ones column of v (softmax denominator accumulator)
    for b in range(2):
        v_sb = P["v_p"].tile([128, 2 * NKC * 65], BF16, tag=f"v{b}", name=f"v{b}")
        nc.vector.memset(
            v_sb[:].rearrange("p (hj x) -> p hj x", x=65)[:, :, 64:65], 1.0
        )
        P[f"v{b}"] = v_sb


def _v_unit(nc, P, b, js, mybir):
    """v rows for batch b, key chunks js: [128, 65] per (head, kchunk)."""
    F32 = mybir.dt.float32
    xt_sb, wqkv_sb, v_sb = P["xt_sb"], P["wqkv_sb"], P[f"v{b}"]
    for j in js:
        ps = P["aux_p"].tile([128, 512], F32, tag="aux", name="vps")
        for cc in range(CC):
            nc.tensor.matmul(
                ps[:, 0:128],
                xt_sb[:, (b * CC + cc) * T + j * 128 : (b * CC + cc) * T + (j + 1) * 128],
                wqkv_sb[:, cc * WCOL + 256 : cc * WCOL + 384],
                start=(cc == 0),
                stop=(cc == CC - 1),
            )
        nc.vector.tensor_copy(
            v_sb[:].rearrange("p (hj x) -> p hj x", x=65)[:, j::NKC, 0:64],
            ps[:, 0:128].rearrange("p (h d) -> p h d", d=64),
        )


def _qk_unit(nc, P, b, kind, tb, nh, mybir):
    """One 512-col block of q^T or k^T for batch b (8 matmuls + 1 copy)."""
    F32 = mybir.dt.float32
    xt_sb, wqkv_sb = P["xt_sb"], P["wqkv_sb"]
    dst = P["qt"][b] if kind == 0 else P["kt"][b]
    mcol = kind * 128
    t0 = tb * 1024 + nh * 512
    ps = P["aux_p"].tile([128, 512], F32, tag="aux", name="qkps")
    for cc in range(CC):
        nc.tensor.matmul(
            ps[:],
            wqkv_sb[:, cc * WCOL + mcol : cc * WCOL + mcol + 128],
            xt_sb[:, (b * CC + cc) * T + t0 : (b * CC + cc) * T + t0 + 512],
            start=(cc == 0),
            stop=(cc == CC - 1),
        )
    nc.vector.tensor_copy(dst[:, t0 : t0 + 512], ps[:])


def _gath_reads(nc, P, b, mybir):
    """Read pair-block columns of batch b's AllToAll output (sync queue)."""
    gath = P[f"gath{b}"]
    den = P[f"den{b}"]
    gat = P[f"gat{b}"]
    gv = gath.rearrange("(s p f) -> p s f", s=8, p=130)
    # dens: row 64 (hh=0) of each shard -> den rows 0:8, row 129 -> rows 8:16
    nc.sync.dma_start(
        den[0:8, :], gv[64:65, :, :].rearrange("p s f -> (p s) f")
    )
    nc.sync.dma_start(
        den[8:16, :], gv[129:130, :, :].rearrange("p s f -> (p s) f")
    )
    # d-rows: heads 2s at partitions 0:64, heads 2s+1 at 64:128
    nc.sync.dma_start(
        gat[0:64, :].rearrange("p (s f) -> p s f", s=8), gv[0:64, :, :]
    )
    nc.sync.dma_start(
        gat[64:128, :].rearrange("p (s f) -> p s f", s=8), gv[65:129, :, :]
    )


def _recip_unit(nc, P, b, mybir):
    F32 = mybir.dt.float32
    denf = P["sm_p"].tile([16, 256], F32, tag="smf", name=f"denf{b}")
    nc.vector.tensor_copy(denf[:], P[f"den{b}"][:])
    rec = P["sm_p"].tile([16, 256], F32, tag="smf", name=f"rec{b}")
    nc.vector.reciprocal_approx_fast(out=rec[:], in_=denf[:])
    recb = P["sm_p"].tile(
        [16, 256], mybir.dt.bfloat16, tag=f"recb{b}", name=f"recb{b}"
    )
    nc.vector.tensor_copy(recb[:], rec[:])
    P[f"recb{b}"] = recb


def _bc_mult_unit(nc, P, b, mybir):
    """Normalize gat (batch b) in place: PE broadcast of 1/den + DVE mult."""
    F32 = mybir.dt.float32
    gat, sel, recb = P[f"gat{b}"], P["sel"], P[f"recb{b}"]
    for s in range(8):
        bc = P["aux_p"].tile([128, 512], F32, tag="aux", name="bc")
        nc.tensor.matmul(
            bc[:, 0:256], sel[:, s * 128 : (s + 1) * 128], recb[:],
            start=True, stop=True,
        )
        blk = gat[:, s * 256 : (s + 1) * 256]
        nc.vector.tensor_mul(blk, blk, bc[:, 0:256])


def _wo_unit(nc, P, b, thfs, mybir):
    """Wo contraction (full 1024 chan) for y blocks thfs of batch b."""
    F32 = mybir.dt.float32
    gat, wo_sb = P[f"gat{b}"], P["wo_sb"]
    for t, hf in thfs:
        ps = P["aux_p"].tile([128, 512], F32, tag="aux", name="wops")
        for s in range(8):
            nc.tensor.matmul(
                ps[:],
                gat[:, s * 256 + t * 128 : s * 256 + (t + 1) * 128],
                wo_sb[:, s * C + hf * 512 : s * C + (hf + 1) * 512],
                start=(s == 0),
                stop=(s == 7),
            )
        yb = P["y_p"].tile([128, 512], F32, tag="y", name="yb")
        nc.vector.tensor_copy(yb[:], ps[:])
        nc.sync.dma_start(
            P["out_ext"][
                b * 256 + t * 128 : b * 256 + (t + 1) * 128,
                hf * 512 : (hf + 1) * 512,
            ],
            yb[:],
        )


def _attention_qg(nc, P, b, qg, fillers, mybir):
    """Scores^T + exp + AV for batch b's two heads, one query group.

    fillers: list of zero-arg closures emitting background PE work; one is
    drained per jp iteration (after scores/exp, before the pipelined AV),
    leftovers at the end of the group.
    """
    F32, BF16 = mybir.dt.float32, mybir.dt.bfloat16
    AFT = mybir.ActivationFunctionType
    qt, kt, v_sb, mask = P["qt"][b], P["kt"][b], P[f"v{b}"], P["mask"]

    njc = 4 * qg + 4          # key chunks (incl. diagonal) for this block
    avs = [
        P["av_p"].tile([65, 512], F32, tag="av", name=f"av{hh}")
        for hh in range(2)
    ]
    # masks: all batch-1 masks on vector -- the gpsimd queue hosts the A2A0
    # collective instruction, which blocks it until the collective completes
    meng = nc.gpsimd if (b == 0 and qg >= 2) else nc.vector

    def emit_avs(att2, jp):
        for hh in range(2):
            for dj in range(2):
                j = 2 * jp + dj
                lo = max((j - 4 * qg) * 128, 0)
                nc.tensor.matmul(
                    avs[hh][:, lo:],
                    v_sb[:, (hh * NKC + j) * 65 : (hh * NKC + j) * 65 + 65],
                    att2[hh][:, dj * 512 + lo : (dj + 1) * 512],
                    start=(j == 0),
                    stop=(j == njc - 1),
                )

    pend = None  # 1-deep software pipeline: scores(jp+1) before AV(jp)
    for jp in range(njc // 2):
        # interleave the two heads' score matmuls so the (0,0)/(64,0) PE
        # tiles run concurrently
        ps2 = [
            P["mm_p"].tile([128, 1024], F32, tag="mm", name=f"scps{hh}")
            for hh in range(2)
        ]
        for dj in range(2):
            j = 2 * jp + dj
            lo = max((j - 4 * qg) * 128, 0)  # skip sub-causal columns
            for hh in range(2):
                nc.tensor.matmul(
                    ps2[hh][:, dj * 512 + lo : (dj + 1) * 512],
                    kt[hh * 64 : (hh + 1) * 64, j * 128 : (j + 1) * 128],
                    qt[hh * 64 : (hh + 1) * 64, qg * 512 + lo : (qg + 1) * 512],
                    start=True,
                    stop=True,
                    tile_position=(hh * 64, 0),
                )
        st = 256 if jp == njc // 2 - 1 else 0  # last jp: cols<256 sub-causal
        att2 = []
        for hh in range(2):
            a2 = P["att_p"].tile([128, 1024], BF16, tag="att", name="a2")
            nc.scalar.activation(a2[:, st:], ps2[hh][:, st:], AFT.Exp)
            att2.append(a2)
        for dj in range(2):
            j = 2 * jp + dj
            l0 = (j - 4 * qg) * 128
            if l0 >= 0:  # diagonal chunk: triangular 0/1 mask
                for hh in range(2):
                    meng.tensor_mul(
                        att2[hh][:, dj * 512 + l0 : dj * 512 + l0 + 128],
                        att2[hh][:, dj * 512 + l0 : dj * 512 + l0 + 128],
                        mask[:],
                    )
        if fillers:
            fillers.pop(0)()
        if pend is not None:
            emit_avs(*pend)
        pend = (att2, jp)
    for f in fillers:
        f()
    fillers.clear()
    emit_avs(*pend)

    # evacuate unnormalized out^T + den row straight to the bounce shards
    # (the 512-query group spans two owners: shards 2qg and 2qg+1)
    bnc = P[f"bounce{b}"]
    for hh in range(2):
        ob = P["ob_p"].tile([65, 512], BF16, tag="ob", name="ob")
        nc.vector.tensor_copy(ob[:], avs[hh][:])
        for half in range(2):
            sh = 2 * qg + half
            nc.sync.dma_start(
                bnc[sh * SH + hh * HOFF : sh * SH + hh * HOFF + HOFF].rearrange(
                    "(q f) -> q f", q=65
                ),
                ob[:, half * 256 : (half + 1) * 256],
            )


def _heartbeat(nc, P, steps, mybir):
    """Dep-chained tiny MM + DVE copy ladder: keeps the PE's HAM clock
    warm across a collective wait without delaying real work much."""
    F32, BF16 = mybir.dt.float32, mybir.dt.bfloat16
    hb = [
        P["const_p"].tile([16, 16], BF16, tag=f"hb{i}", name=f"hb{i}")
        for i in range(2)
    ]
    nc.vector.memset(hb[0][:], 0.25)
    for k in range(steps):
        ps = P["mm_p"].tile([128, 1024], F32, tag="mm", name="hbps")
        nc.tensor.matmul(
            ps[0:16, 0:16], hb[k % 2][:], hb[k % 2][:], start=True, stop=True
        )
        nc.vector.tensor_copy(hb[(k + 1) % 2][:], ps[0:16, 0:16])


def _a2a(nc, P, b, mybir):
    """AllToAll batch b's bounced shards across all 8 cores."""
    groups = [[0, 1, 2, 3, 4, 5, 6, 7]]
    nc.gpsimd.collective_compute(
        "AllToAll", mybir.AluOpType.bypass, replica_groups=groups,
        ins=[P[f"bounce{b}"].opt()], outs=[P[f"gath{b}"].opt()],
    )


def _body(nc, P, mybir):
    F32, BF16 = mybir.dt.float32, mybir.dt.bfloat16
    _loads(nc, P, mybir)

    for b in range(2):
        qt_t = P["qk_p"].tile([128, T], BF16, tag=f"qt{b}", name=f"qt{b}")
        kt_t = P["qk_p"].tile([128, T], BF16, tag=f"kt{b}", name=f"kt{b}")
        P.setdefault("qt", []).append(qt_t)
        P.setdefault("kt", []).append(kt_t)
        P[f"gat{b}"] = P["gat_p"].tile(
            [128, 8 * OWN], BF16, tag=f"gat{b}", name=f"gat{b}"
        )
        P[f"den{b}"] = P["sm_p"].tile(
            [16, 256], BF16, tag=f"den{b}", name=f"den{b}"
        )
        P[f"bounce{b}"] = P["dram_p"].tile(
            [8 * SH], BF16, tag=f"bounce{b}", name=f"bounce{b}"
        )
        P[f"gath{b}"] = P["dram_p"].tile(
            [8 * SH], BF16, tag=f"gath{b}", name=f"gath{b}"
        )

    U = lambda *a: (lambda: _qk_unit(nc, P, *a, mybir))
    V = lambda b, *js: (lambda: _v_unit(nc, P, b, js, mybir))
    WO = lambda b, *thfs: (lambda: _wo_unit(nc, P, b, thfs, mybir))

    # lead-in: just enough q^T/k^T/v for batch-0 qg0
    _qk_unit(nc, P, 0, 1, 0, 0, mybir)
    _qk_unit(nc, P, 0, 0, 0, 0, mybir)
    _v_unit(nc, P, 0, [0, 1], mybir)

    _attention_qg(nc, P, 0, 0, [V(0, 2, 3), U(0, 1, 0, 1), U(0, 0, 0, 1)], mybir)
    _attention_qg(
        nc, P, 0, 1, [V(0, 4, 5), V(0, 6, 7), U(0, 1, 1, 0), U(0, 0, 1, 0)], mybir
    )
    _attention_qg(
        nc, P, 0, 2,
        [V(0, 8, 9), V(0, 10, 11), U(0, 1, 1, 1), U(0, 0, 1, 1),
         U(1, 1, 0, 0), U(1, 0, 0, 0)],
        mybir,
    )
    _attention_qg(
        nc, P, 0, 3,
        [V(0, 12, 13), V(0, 14, 15), V(1, 0, 1), V(1, 2, 3),
         U(1, 1, 0, 1), U(1, 0, 0, 1), U(1, 1, 1, 0), U(1, 0, 1, 0)],
        mybir,
    )
    _a2a(nc, P, 0, mybir)

    _attention_qg(nc, P, 1, 0, [V(1, 4, 5), V(1, 6, 7)], mybir)
    _attention_qg(
        nc, P, 1, 1,
        [V(1, 8, 9), V(1, 10, 11), U(1, 1, 1, 1), U(1, 0, 1, 1)],
        mybir,
    )
    # A2A0-dependent work sits deep in qg2 so the DVE queue never stalls
    # on the collective (sync-queue gath reads absorb any remaining wait)
    _attention_qg(
        nc, P, 1, 2,
        [lambda: _gath_reads(nc, P, 0, mybir),
         V(1, 12, 13), V(1, 14, 15),
         lambda: _recip_unit(nc, P, 0, mybir),
         lambda: _bc_mult_unit(nc, P, 0, mybir),
         WO(0, (0, 0))],
        mybir,
    )
    _attention_qg(nc, P, 1, 3, [WO(0, (0, 1)), WO(0, (1, 0))], mybir)
    _a2a(nc, P, 1, mybir)

    # fill the A2A1 wait: last wo0 block, then a HAM-warming heartbeat
    _wo_unit(nc, P, 0, [(1, 1)], mybir)
    _heartbeat(nc, P, 20, mybir)
    _gath_reads(nc, P, 1, mybir)
    _recip_unit(nc, P, 1, mybir)
    _bc_mult_unit(nc, P, 1, mybir)
    _wo_unit(nc, P, 1, [(0, 0), (0, 1)], mybir)
    _wo_unit(nc, P, 1, [(1, 0), (1, 1)], mybir)


def _build():
    import concourse.mybir as mybir
    import concourse.tile as tile
    from concourse import bacc

    F32, BF16 = mybir.dt.float32, mybir.dt.bfloat16

    nc = bacc.Bacc("TRN2", target_bir_lowering=False, debug=False, num_devices=8)
    P = {
        "xt_ext": nc.declare_dram_parameter("xt", [2 * C, T], BF16, isOutput=False),
        "wqkv_ext": nc.declare_dram_parameter("wqkv", [C, WCOL], BF16, isOutput=False),
        "wo_ext": nc.declare_dram_parameter("wo", [C, C], BF16, isOutput=False),
        "mask_ext": nc.declare_dram_parameter("mask", [128, 128], BF16, isOutput=False),
        "sel_ext": nc.declare_dram_parameter("sel", [16, 1024], BF16, isOutput=False),
        "out_ext": nc.declare_dram_parameter("out", [2 * OWN, C], F32, isOutput=True),
    }

    with tile.TileContext(nc) as tc:
        with (
            tc.tile_pool(name="const", bufs=1) as const_p,
            tc.tile_pool(name="w", bufs=1) as w_p,
            tc.tile_pool(name="x", bufs=1) as x_p,
            tc.tile_pool(name="qk", bufs=1) as qk_p,
            tc.tile_pool(name="v", bufs=1) as v_p,
            tc.tile_pool(name="att", bufs=4) as att_p,
            tc.tile_pool(name="ob", bufs=4) as ob_p,
            tc.tile_pool(name="gat", bufs=1) as gat_p,
            tc.tile_pool(name="y", bufs=2) as y_p,
            tc.tile_pool(name="sm", bufs=2) as sm_p,
            tc.tile_pool(name="mm", bufs=2, space="PSUM") as mm_p,
            tc.tile_pool(name="av", bufs=2, space="PSUM") as av_p,
            tc.tile_pool(name="aux", bufs=2, space="PSUM") as aux_p,
            tc.tile_pool(name="dram", bufs=1, space="DRAM") as dram_p,
        ):
            P.update(
                const_p=const_p, w_p=w_p, x_p=x_p, qk_p=qk_p, v_p=v_p,
                att_p=att_p, ob_p=ob_p, gat_p=gat_p, y_p=y_p, sm_p=sm_p,
                mm_p=mm_p, av_p=av_p, aux_p=aux_p, dram_p=dram_p,
            )
            _body(nc, P, mybir)

    nc.finalize()
    return nc


def kernel(x, Wqkv, bqkv, Wo, bo):
    global _cached_nc, last_result
    import ml_dtypes
    from concourse.bass_utils import run_bass_kernel_spmd

    if _cached_nc is None:
        _cached_nc = _build()
    nc = _cached_nc

    bf16 = ml_dtypes.bfloat16
    x = np.asarray(x, dtype=np.float32)
    Wqkv = np.asarray(Wqkv, dtype=np.float32)
    wo_b = np.ascontiguousarray(np.asarray(Wo, dtype=np.float32).astype(bf16))

    # x^T for both batches stacked: rows [b*C + c]
    xt = np.ascontiguousarray(
        np.concatenate([x[0].T, x[1].T], axis=0).astype(bf16)
    )

    # lower-triangle 0/1 mask for diagonal blocks: partition = key, free = query
    tri = (np.arange(128)[:, None] <= np.arange(128)[None, :]).astype(bf16)
    tri = np.ascontiguousarray(tri)

    # selection matrix for the 1/den PE broadcast: chunk s rows 0:64 get
    # head 2s's den (sel row s), rows 64:128 get head 2s+1's (row 8+s)
    sel = np.zeros((16, 1024), dtype=bf16)
    for s in range(8):
        sel[s, s * 128 : s * 128 + 64] = 1
        sel[8 + s, s * 128 + 64 : s * 128 + 128] = 1
    sel = np.ascontiguousarray(sel)

    in_maps = []
    for core in range(8):
        c0 = core * 2 * D
        wq = Wqkv[:, c0 : c0 + 128] * SCALE
        wk = Wqkv[:, C + c0 : C + c0 + 128]
        wv = Wqkv[:, 2 * C + c0 : 2 * C + c0 + 128]
        wqkv = np.ascontiguousarray(
            np.concatenate([wq, wk, wv], axis=1).astype(bf16)
        )
        in_maps.append(
            {"xt": xt, "wqkv": wqkv, "wo": wo_b, "mask": tri, "sel": sel}
        )

    last_result = run_bass_kernel_spmd(nc, in_maps, core_ids=list(range(8)))

    y = np.empty((B, T, C), dtype=np.float32)
    for core in range(8):
        r = last_result.results[core]["out"]
        y[0, core * OWN : (core + 1) * OWN, :] = r[0:OWN]
        y[1, core * OWN : (core + 1) * OWN, :] = r[OWN : 2 * OWN]
    return y


# revision 14
# speedup vs baseline: 1.5060x; 1.0341x over previous
"""Distributed causal multi-head attention (Bass/Tile, 8 TRN2 NeuronCores).

Sharding: core c owns heads (2c, 2c+1) of BOTH batches, and owns output
rows [c*256, (c+1)*256) of each batch.  QKV for its 2 heads is computed
from the full x (both batches) locally -- no K/V collective.  After each
batch's attention, one 8-rank AllToAll redistributes UNNORMALIZED
attention outputs plus the softmax denominator row (ones-column trick)
to the query-row owners; each core then normalizes (batched fast
reciprocal + PE broadcast) and applies the full Wo to its rows.

Per core, per batch b:
  q^T, k^T = (x_b @ Wq/Wk)^T  [128, 2048]  (partitions = 2 heads x 64 dims)
  v        =  x_b @ Wv        [128, 65] per (head, kchunk), ones col appended
  scores^T = k^T.T @ q^T      head-interleaved pairs run concurrently on PE
  softmax: full-width exp (scalar engine), causal mask as 0/1 post-mult,
  AV matmul with ones column -> unnormalized out^T + denominator row,
  AllToAll (bf16) across all 8 cores, normalize after gather, y = out^T.T @ Wo.

All projection / v / Wo matmul work is chopped into ~1-2us units and
drained into the attention loop between exp-paced iterations so the PE
stays dense (HAM clock-gate stays warm).
"""

import numpy as np

B, T, C, H = 2, 2048, 1024, 16
D = C // H            # 64
NQG = T // 512        # 4 query groups of 512
NKC = T // 128        # 16 key chunks of 128
CC = C // 128         # 8 contraction chunks
WCOL = 3 * 2 * D      # 384 packed qkv columns per core (2 heads)
SCALE = 1.0 / 32.0    # 1/sqrt(C), folded into Wq on host
OWN = 256             # query rows owned per (core, batch)
SH = 130 * OWN        # AllToAll shard elems: 2 heads x (64 d + den) x 256 q
HOFF = 65 * OWN       # head hh=1 offset inside a shard

_cached_nc = None
last_result = None


def _loads(nc, P, mybir):
    from concourse.bass import ts

    F32, BF16 = mybir.dt.float32, mybir.dt.bfloat16
    AFT = mybir.ActivationFunctionType

    # small consts first on sync
    mask = P["const_p"].tile([128, 128], BF16, tag="mask")
    nc.sync.dma_start(mask[:], P["mask_ext"][:])
    P["mask"] = mask
    sel = P["const_p"].tile([16, 1024], BF16, tag="sel", name="sel")
    nc.sync.dma_start(sel[:], P["sel_ext"][:])
    P["sel"] = sel

    # packed qkv weights first on scalar (before any exp is emitted)
    wqkv_sb = P["w_p"].tile([128, CC * WCOL], BF16, tag="wqkv")
    for cc in range(CC):
        nc.scalar.dma_start(
            wqkv_sb[:, cc * WCOL : (cc + 1) * WCOL], P["wqkv_ext"][ts(cc, 128), :]
        )
    P["wqkv_sb"] = wqkv_sb

    # x^T for both batches, column halves (batch0 cols 0:1024 first).
    # The first half-batch is striped across sync+scalar so the lead-in
    # q/k units can start ~6us in.  (gpsimd can't host early DMAs: the
    # collective entry barrier occupies that queue at kernel start.)
    xt_sb = P["x_p"].tile([128, 2 * CC * T], BF16, tag="xt")
    P["xt_sb"] = xt_sb
    order = [(0, 0), (0, 1), (1, 0), (1, 1)]
    for b, th in order:
        for cc in range(CC):
            q = nc.scalar if (b, th) == (0, 0) and cc % 2 == 1 else nc.sync
            q.dma_start(
                xt_sb[:, (b * CC + cc) * T + th * 1024 :
                      (b * CC + cc) * T + (th + 1) * 1024],
                P["xt_ext"][b * C + cc * 128 : b * C + (cc + 1) * 128,
                            th * 1024 : (th + 1) * 1024],
            )

    # warm the ACT exp table while DMAs stream
    scr = P["const_p"].tile([128, 32], F32, tag="scr", name="scr")
    nc.vector.memset(scr[:, 0:16], 0.0)
    nc.scalar.activation(scr[:, 16:32], scr[:, 0:16], AFT.Exp)

    # Wo on sync behind x^T (needed only ~60us in)
    wo_sb = P["w_p"].tile([128, CC * C], BF16, tag="wo")
    for cc in range(CC):
        nc.sync.dma_start(wo_sb[:, cc * C : (cc + 1) * C], P["wo_ext"][ts(cc, 128), :])
    P["wo_sb"] = wo_sb

    # ones column of v (softmax denominator accumulator)
    for b in range(2):
        v_sb = P["v_p"].tile([128, 2 * NKC * 65], BF16, tag=f"v{b}", name=f"v{b}")
        nc.vector.memset(
            v_sb[:].rearrange("p (hj x) -> p hj x", x=65)[:, :, 64:65], 1.0
        )
        P[f"v{b}"] = v_sb


def _v_unit(nc, P, b, js, mybir):
    """v rows for batch b, key chunks js: [128, 65] per (head, kchunk)."""
    F32 = mybir.dt.float32
    xt_sb, wqkv_sb, v_sb = P["xt_sb"], P["wqkv_sb"], P[f"v{b}"]
    for j in js:
        ps = P["aux_p"].tile([128, 512], F32, tag="aux", name="vps")
        for cc in range(CC):
            nc.tensor.matmul(
                ps[:, 0:128],
                xt_sb[:, (b * CC + cc) * T + j * 128 : (b * CC + cc) * T + (j + 1) * 128],
                wqkv_sb[:, cc * WCOL + 256 : cc * WCOL + 384],
                start=(cc == 0),
                stop=(cc == CC - 1),
            )
        nc.vector.tensor_copy(
            v_sb[:].rearrange("p (hj x) -> p hj x", x=65)[:, j::NKC, 0:64],
            ps[:, 0:128].rearrange("p (h d) -> p h d", d=64),
        )


def _qk_unit(nc, P, b, kind, tb, nh, mybir):
    """One 512-col block of q^T or k^T for batch b (8 matmuls + 1 copy)."""
    F32 = mybir.dt.float32
    xt_sb, wqkv_sb = P["xt_sb"], P["wqkv_sb"]
    dst = P["qt"][b] if kind == 0 else P["kt"][b]
    mcol = kind * 128
    t0 = tb * 1024 + nh * 512
    ps = P["aux_p"].tile([128, 512], F32, tag="aux", name="qkps")
    for cc in range(CC):
        nc.tensor.matmul(
            ps[:],
            wqkv_sb[:, cc * WCOL + mcol : cc * WCOL + mcol + 128],
            xt_sb[:, (b * CC + cc) * T + t0 : (b * CC + cc) * T + t0 + 512],
            start=(cc == 0),
            stop=(cc == CC - 1),
        )
    nc.vector.tensor_copy(dst[:, t0 : t0 + 512], ps[:])


def _gath_reads(nc, P, b, mybir):
    """Read pair-block columns of batch b's AllToAll output (sync queue)."""
    gath = P[f"gath{b}"]
    den = P[f"den{b}"]
    gat = P[f"gat{b}"]
    gv = gath.rearrange("(s p f) -> p s f", s=8, p=130)
    # dens: row 64 (hh=0) of each shard -> den rows 0:8, row 129 -> rows 8:16
    nc.sync.dma_start(
        den[0:8, :], gv[64:65, :, :].rearrange("p s f -> (p s) f")
    )
    nc.sync.dma_start(
        den[8:16, :], gv[129:130, :, :].rearrange("p s f -> (p s) f")
    )
    # d-rows: heads 2s at partitions 0:64, heads 2s+1 at 64:128
    nc.sync.dma_start(
        gat[0:64, :].rearrange("p (s f) -> p s f", s=8), gv[0:64, :, :]
    )
    nc.sync.dma_start(
        gat[64:128, :].rearrange("p (s f) -> p s f", s=8), gv[65:129, :, :]
    )


def _recip_unit(nc, P, b, mybir):
    F32 = mybir.dt.float32
    denf = P["sm_p"].tile([16, 256], F32, tag="smf", name=f"denf{b}")
    nc.vector.tensor_copy(denf[:], P[f"den{b}"][:])
    rec = P["sm_p"].tile([16, 256], F32, tag="smf", name=f"rec{b}")
    nc.vector.reciprocal_approx_fast(out=rec[:], in_=denf[:])
    recb = P["sm_p"].tile(
        [16, 256], mybir.dt.bfloat16, tag=f"recb{b}", name=f"recb{b}"
    )
    nc.vector.tensor_copy(recb[:], rec[:])
    P[f"recb{b}"] = recb


def _bc_mult_unit(nc, P, b, mybir):
    """Normalize gat (batch b) in place: PE broadcast of 1/den + DVE mult."""
    F32 = mybir.dt.float32
    gat, sel, recb = P[f"gat{b}"], P["sel"], P[f"recb{b}"]
    for s in range(8):
        bc = P["aux_p"].tile([128, 512], F32, tag="aux", name="bc")
        nc.tensor.matmul(
            bc[:, 0:256], sel[:, s * 128 : (s + 1) * 128], recb[:],
            start=True, stop=True,
        )
        blk = gat[:, s * 256 : (s + 1) * 256]
        nc.vector.tensor_mul(blk, blk, bc[:, 0:256])


def _wo_unit(nc, P, b, thfs, mybir):
    """Wo contraction (full 1024 chan) for y blocks thfs of batch b."""
    F32 = mybir.dt.float32
    gat, wo_sb = P[f"gat{b}"], P["wo_sb"]
    for t, hf in thfs:
        ps = P["aux_p"].tile([128, 512], F32, tag="aux", name="wops")
        for s in range(8):
            nc.tensor.matmul(
                ps[:],
                gat[:, s * 256 + t * 128 : s * 256 + (t + 1) * 128],
                wo_sb[:, s * C + hf * 512 : s * C + (hf + 1) * 512],
                start=(s == 0),
                stop=(s == 7),
            )
        yb = P["y_p"].tile([128, 512], F32, tag="y", name="yb")
        nc.vector.tensor_copy(yb[:], ps[:])
        nc.sync.dma_start(
            P["out_ext"][
                b * 256 + t * 128 : b * 256 + (t + 1) * 128,
                hf * 512 : (hf + 1) * 512,
            ],
            yb[:],
        )


def _attention_qg(nc, P, b, qg, fillers, mybir):
    """Scores^T + exp + AV for batch b's two heads, one query group.

    fillers: list of zero-arg closures emitting background PE work; one is
    drained per jp iteration (after scores/exp, before the pipelined AV),
    leftovers at the end of the group.
    """
    F32, BF16 = mybir.dt.float32, mybir.dt.bfloat16
    AFT = mybir.ActivationFunctionType
    qt, kt, v_sb, mask = P["qt"][b], P["kt"][b], P[f"v{b}"], P["mask"]

    njc = 4 * qg + 4          # key chunks (incl. diagonal) for this block
    avs = [
        P["av_p"].tile([65, 512], F32, tag="av", name=f"av{hh}")
        for hh in range(2)
    ]
    # masks: all batch-1 masks on vector -- the gpsimd queue hosts the A2A0
    # collective instruction, which blocks it until the collective completes
    meng = nc.gpsimd if (b == 0 and qg >= 2) else nc.vector

    def emit_avs(att2, jp):
        for hh in range(2):
            for dj in range(2):
                j = 2 * jp + dj
                lo = max((j - 4 * qg) * 128, 0)
                nc.tensor.matmul(
                    avs[hh][:, lo:],
                    v_sb[:, (hh * NKC + j) * 65 : (hh * NKC + j) * 65 + 65],
                    att2[hh][:, dj * 512 + lo : (dj + 1) * 512],
                    start=(j == 0),
                    stop=(j == njc - 1),
                )

    pend = None  # 1-deep software pipeline: scores(jp+1) before AV(jp)
    for jp in range(njc // 2):
        # interleave the two heads' score matmuls so the (0,0)/(64,0) PE
        # tiles run concurrently
        ps2 = [
            P["mm_p"].tile([128, 1024], F32, tag="mm", name=f"scps{hh}")
            for hh in range(2)
        ]
        for dj in range(2):
            j = 2 * jp + dj
            lo = max((j - 4 * qg) * 128, 0)  # skip sub-causal columns
            for hh in range(2):
                nc.tensor.matmul(
                    ps2[hh][:, dj * 512 + lo : (dj + 1) * 512],
                    kt[hh * 64 : (hh + 1) * 64, j * 128 : (j + 1) * 128],
                    qt[hh * 64 : (hh + 1) * 64, qg * 512 + lo : (qg + 1) * 512],
                    start=True,
                    stop=True,
                    tile_position=(hh * 64, 0),
                )
        st = 256 if jp == njc // 2 - 1 else 0  # last jp: cols<256 sub-causal
        att2 = []
        for hh in range(2):
            a2 = P["att_p"].tile([128, 1024], BF16, tag="att", name="a2")
            nc.scalar.activation(a2[:, st:], ps2[hh][:, st:], AFT.Exp)
            att2.append(a2)
        for dj in range(2):
            j = 2 * jp + dj
            l0 = (j - 4 * qg) * 128
            if l0 >= 0:  # diagonal chunk: triangular 0/1 mask
                for hh in range(2):
                    meng.tensor_mul(
                        att2[hh][:, dj * 512 + l0 : dj * 512 + l0 + 128],
                        att2[hh][:, dj * 512 + l0 : dj * 512 + l0 + 128],
                        mask[:],
                    )
        if fillers and (len(fillers) > 1 or jp < njc // 2 - 1):
            fillers.pop(0)()
        if pend is not None:
            emit_avs(*pend)
        pend = (att2, jp)
    while len(fillers) > 1:
        fillers.pop(0)()
    emit_avs(*pend)
    # one filler held back to cover the PE while the final AV chain and
    # the avs evacuation drain (kills the ~2.4us qg-boundary bubble)
    for f in fillers:
        f()
    fillers.clear()

    # evacuate unnormalized out^T + den row straight to the bounce shards
    # (the 512-query group spans two owners: shards 2qg and 2qg+1)
    bnc = P[f"bounce{b}"]
    for hh in range(2):
        ob = P["ob_p"].tile([65, 512], BF16, tag="ob", name="ob")
        nc.vector.tensor_copy(ob[:], avs[hh][:])
        for half in range(2):
            sh = 2 * qg + half
            nc.sync.dma_start(
                bnc[sh * SH + hh * HOFF : sh * SH + hh * HOFF + HOFF].rearrange(
                    "(q f) -> q f", q=65
                ),
                ob[:, half * 256 : (half + 1) * 256],
            )


def _heartbeat(nc, P, steps, mybir):
    """Dep-chained tiny MM + DVE copy ladder: keeps the PE's HAM clock
    warm across a collective wait without delaying real work much."""
    F32, BF16 = mybir.dt.float32, mybir.dt.bfloat16
    hb = [
        P["const_p"].tile([16, 16], BF16, tag=f"hb{i}", name=f"hb{i}")
        for i in range(2)
    ]
    nc.vector.memset(hb[0][:], 0.25)
    for k in range(steps):
        ps = P["mm_p"].tile([128, 1024], F32, tag="mm", name="hbps")
        nc.tensor.matmul(
            ps[0:16, 0:16], hb[k % 2][:], hb[k % 2][:], start=True, stop=True
        )
        nc.vector.tensor_copy(hb[(k + 1) % 2][:], ps[0:16, 0:16])


def _a2a(nc, P, b, mybir):
    """AllToAll batch b's bounced shards across all 8 cores."""
    groups = [[0, 1, 2, 3, 4, 5, 6, 7]]
    nc.gpsimd.collective_compute(
        "AllToAll", mybir.AluOpType.bypass, replica_groups=groups,
        ins=[P[f"bounce{b}"].opt()], outs=[P[f"gath{b}"].opt()],
    )


def _body(nc, P, mybir):
    F32, BF16 = mybir.dt.float32, mybir.dt.bfloat16
    _loads(nc, P, mybir)

    for b in range(2):
        qt_t = P["qk_p"].tile([128, T], BF16, tag=f"qt{b}", name=f"qt{b}")
        kt_t = P["qk_p"].tile([128, T], BF16, tag=f"kt{b}", name=f"kt{b}")
        P.setdefault("qt", []).append(qt_t)
        P.setdefault("kt", []).append(kt_t)
        P[f"gat{b}"] = P["gat_p"].tile(
            [128, 8 * OWN], BF16, tag=f"gat{b}", name=f"gat{b}"
        )
        P[f"den{b}"] = P["sm_p"].tile(
            [16, 256], BF16, tag=f"den{b}", name=f"den{b}"
        )
        P[f"bounce{b}"] = P["dram_p"].tile(
            [8 * SH], BF16, tag=f"bounce{b}", name=f"bounce{b}"
        )
        P[f"gath{b}"] = P["dram_p"].tile(
            [8 * SH], BF16, tag=f"gath{b}", name=f"gath{b}"
        )

    U = lambda *a: (lambda: _qk_unit(nc, P, *a, mybir))
    V = lambda b, *js: (lambda: _v_unit(nc, P, b, js, mybir))
    WO = lambda b, *thfs: (lambda: _wo_unit(nc, P, b, thfs, mybir))

    # lead-in: just enough q^T/k^T/v for batch-0 qg0
    _qk_unit(nc, P, 0, 1, 0, 0, mybir)
    _qk_unit(nc, P, 0, 0, 0, 0, mybir)
    _v_unit(nc, P, 0, [0, 1], mybir)

    _attention_qg(nc, P, 0, 0, [V(0, 2, 3), U(0, 1, 0, 1), U(0, 0, 0, 1)], mybir)
    _attention_qg(
        nc, P, 0, 1, [V(0, 4, 5), V(0, 6, 7), U(0, 1, 1, 0), U(0, 0, 1, 0)], mybir
    )
    _attention_qg(
        nc, P, 0, 2,
        [V(0, 8, 9), V(0, 10, 11), U(0, 1, 1, 1), U(0, 0, 1, 1),
         U(1, 1, 0, 0), U(1, 0, 0, 0)],
        mybir,
    )
    _attention_qg(
        nc, P, 0, 3,
        [V(0, 12, 13), V(0, 14, 15), V(1, 0, 1), V(1, 2, 3),
         U(1, 1, 0, 1), U(1, 0, 0, 1), U(1, 1, 1, 0), U(1, 0, 1, 0)],
        mybir,
    )
    _a2a(nc, P, 0, mybir)

    _attention_qg(nc, P, 1, 0, [V(1, 4, 5), V(1, 6, 7)], mybir)
    _attention_qg(
        nc, P, 1, 1,
        [V(1, 8, 9), V(1, 10, 11), U(1, 1, 1, 1), U(1, 0, 1, 1)],
        mybir,
    )
    # A2A0-dependent work sits deep in qg2 so the DVE queue never stalls
    # on the collective (sync-queue gath reads absorb any remaining wait)
    _attention_qg(
        nc, P, 1, 2,
        [lambda: _gath_reads(nc, P, 0, mybir),
         V(1, 12, 13), V(1, 14, 15),
         lambda: _recip_unit(nc, P, 0, mybir),
         lambda: _bc_mult_unit(nc, P, 0, mybir),
         WO(0, (0, 0))],
        mybir,
    )
    _attention_qg(nc, P, 1, 3, [WO(0, (0, 1)), WO(0, (1, 0))], mybir)
    _a2a(nc, P, 1, mybir)

    # fill the A2A1 wait: last wo0 block, then a HAM-warming heartbeat
    _wo_unit(nc, P, 0, [(1, 1)], mybir)
    _heartbeat(nc, P, 40, mybir)
    _gath_reads(nc, P, 1, mybir)
    _recip_unit(nc, P, 1, mybir)
    _bc_mult_unit(nc, P, 1, mybir)
    _wo_unit(nc, P, 1, [(0, 0), (0, 1)], mybir)
    _wo_unit(nc, P, 1, [(1, 0), (1, 1)], mybir)


def _build():
    import concourse.mybir as mybir
    import concourse.tile as tile
    from concourse import bacc

    F32, BF16 = mybir.dt.float32, mybir.dt.bfloat16

    nc = bacc.Bacc("TRN2", target_bir_lowering=False, debug=False, num_devices=8)
    P = {
        "xt_ext": nc.declare_dram_parameter("xt", [2 * C, T], BF16, isOutput=False),
        "wqkv_ext": nc.declare_dram_parameter("wqkv", [C, WCOL], BF16, isOutput=False),
        "wo_ext": nc.declare_dram_parameter("wo", [C, C], BF16, isOutput=False),
        "mask_ext": nc.declare_dram_parameter("mask", [128, 128], BF16, isOutput=False),
        "sel_ext": nc.declare_dram_parameter("sel", [16, 1024], BF16, isOutput=False),
        "out_ext": nc.declare_dram_parameter("out", [2 * OWN, C], F32, isOutput=True),
    }

    with tile.TileContext(nc) as tc:
        with (
            tc.tile_pool(name="const", bufs=1) as const_p,
            tc.tile_pool(name="w", bufs=1) as w_p,
            tc.tile_pool(name="x", bufs=1) as x_p,
            tc.tile_pool(name="qk", bufs=1) as qk_p,
            tc.tile_pool(name="v", bufs=1) as v_p,
            tc.tile_pool(name="att", bufs=4) as att_p,
            tc.tile_pool(name="ob", bufs=4) as ob_p,
            tc.tile_pool(name="gat", bufs=1) as gat_p,
            tc.tile_pool(name="y", bufs=2) as y_p,
            tc.tile_pool(name="sm", bufs=2) as sm_p,
            tc.tile_pool(name="mm", bufs=2, space="PSUM") as mm_p,
            tc.tile_pool(name="av", bufs=2, space="PSUM") as av_p,
            tc.tile_pool(name="aux", bufs=2, space="PSUM") as aux_p,
            tc.tile_pool(name="dram", bufs=1, space="DRAM") as dram_p,
        ):
            P.update(
                const_p=const_p, w_p=w_p, x_p=x_p, qk_p=qk_p, v_p=v_p,
                att_p=att_p, ob_p=ob_p, gat_p=gat_p, y_p=y_p, sm_p=sm_p,
                mm_p=mm_p, av_p=av_p, aux_p=aux_p, dram_p=dram_p,
            )
            _body(nc, P, mybir)

    nc.finalize()
    return nc


def kernel(x, Wqkv, bqkv, Wo, bo):
    global _cached_nc, last_result
    import ml_dtypes
    from concourse.bass_utils import run_bass_kernel_spmd

    if _cached_nc is None:
        _cached_nc = _build()
    nc = _cached_nc

    bf16 = ml_dtypes.bfloat16
    x = np.asarray(x, dtype=np.float32)
    Wqkv = np.asarray(Wqkv, dtype=np.float32)
    wo_b = np.ascontiguousarray(np.asarray(Wo, dtype=np.float32).astype(bf16))

    # x^T for both batches stacked: rows [b*C + c]
    xt = np.ascontiguousarray(
        np.concatenate([x[0].T, x[1].T], axis=0).astype(bf16)
    )

    # lower-triangle 0/1 mask for diagonal blocks: partition = key, free = query
    tri = (np.arange(128)[:, None] <= np.arange(128)[None, :]).astype(bf16)
    tri = np.ascontiguousarray(tri)

    # selection matrix for the 1/den PE broadcast: chunk s rows 0:64 get
    # head 2s's den (sel row s), rows 64:128 get head 2s+1's (row 8+s)
    sel = np.zeros((16, 1024), dtype=bf16)
    for s in range(8):
        sel[s, s * 128 : s * 128 + 64] = 1
        sel[8 + s, s * 128 + 64 : s * 128 + 128] = 1
    sel = np.ascontiguousarray(sel)

    in_maps = []
    for core in range(8):
        c0 = core * 2 * D
        wq = Wqkv[:, c0 : c0 + 128] * SCALE
        wk = Wqkv[:, C + c0 : C + c0 + 128]
        wv = Wqkv[:, 2 * C + c0 : 2 * C + c0 + 128]
        wqkv = np.ascontiguousarray(
            np.concatenate([wq, wk, wv], axis=1).astype(bf16)
        )
        in_maps.append(
            {"xt": xt, "wqkv": wqkv, "wo": wo_b, "mask": tri, "sel": sel}
        )

    last_result = run_bass_kernel_spmd(nc, in_maps, core_ids=list(range(8)))

    y = np.empty((B, T, C), dtype=np.float32)
    for core in range(8):
        r = last_result.results[core]["out"]
        y[0, core * OWN : (core + 1) * OWN, :] = r[0:OWN]
        y[1, core * OWN : (core + 1) * OWN, :] = r[OWN : 2 * OWN]
    return y
